# revision 34
# baseline (speedup 1.0000x reference)
"""Multi-head attention (pre-LN + residual) on 8 trn2 NeuronCores.

Sharding: core r = (batch b = r//4, head group i = r%4, 4 heads each).

Per core: LN(x_b) -> x_norm^T via DMA-crossbar transpose (fp8 copy on
gpsimd) -> fp8 DoubleRow K/Q/V projections (weights pre-scaled x32 on
host; Q/K kept x32 in bf16, V x32 in fp8) -> scores^T = K Q^T per head
(bf16, 64-contraction) -> exp split between ScalarE (exact, fp8 out)
and VectorE (exp2 bit-trick writing fp8e4 bit patterns via uint8
round+saturate) -> AV in fp8 DoubleRow over kt-pairs with a ones
column giving the softmax denominator -> normalize on gpsimd -> fp8
AllToAll per head pair -> fp8 DoubleRow w_o matmul (+1/1024 rescale
fused with bias+residual add) for a 256-row seq slice of both batches.
"""

import sys

sys.path.insert(0, "/opt/trn_rl_repo")

import numpy as np
import ml_dtypes

BF16 = ml_dtypes.bfloat16
F8 = ml_dtypes.float8_e4m3fn

# Problem constants (hardcoded per contract)
B = 2
S = 2048
D = 1024
H = 16
DK = 64
NCORES = 8
HLOC = 4  # heads per core
DLOC = HLOC * DK  # 256
SLICE = S // NCORES  # 256 output rows per batch per core
EPS = 1e-5
WS = 32.0  # fp8 weight scale (avoids e4m3 subnormals)
# scores psum = (32Q)(32K) = 1024*QK ; softmax wants exp(QK/8)
C_EXP = 1.0 / (8.0 * WS * WS)
LOG2E = 1.4426950408889634
CORR = -0.045
A8 = C_EXP * LOG2E * 8.0
B8 = (7.0 + CORR) * 8.0

_CACHE = {}


def _build(phases="ABCDE"):
    import concourse.bass as bass
    import concourse.mybir as mybir
    import concourse.tile as tile
    from concourse import bacc

    from concourse.masks import make_identity

    dt = mybir.dt
    AF = mybir.ActivationFunctionType
    OP = mybir.AluOpType
    DR = mybir.MatmulPerfMode.DoubleRow

    nc = bacc.Bacc(
        "TRN2",
        target_bir_lowering=False,
        debug=False,
        enable_asserts=False,
        num_devices=NCORES,
    )

    # ---- I/O ----
    x_b = nc.dram_tensor("x_b", [S, D], dt.float32, kind="ExternalInput").ap()
    wq8 = nc.dram_tensor("wq8", [D, DLOC], dt.float8e4, kind="ExternalInput").ap()
    wk8 = nc.dram_tensor("wk8", [D, DLOC], dt.float8e4, kind="ExternalInput").ap()
    wv8 = nc.dram_tensor("wv8", [D, DLOC], dt.float8e4, kind="ExternalInput").ap()
    wo8 = nc.dram_tensor("wo8", [D, D], dt.float8e4, kind="ExternalInput").ap()
    x_res = nc.dram_tensor(
        "x_res", [B, SLICE, D], dt.float32, kind="ExternalInput"
    ).ap()
    b_o = nc.dram_tensor("b_o", [D], dt.float32, kind="ExternalInput").ap()
    out_sl = nc.dram_tensor(
        "out_sl", [B, SLICE, D], dt.float32, kind="ExternalOutput"
    ).ap()
    if "g" in phases:
        xnt_dbg = nc.dram_tensor(
            "xnt_dbg", [128, 4, 8, 512], dt.float8e4, kind="ExternalOutput"
        ).ap()
        qk_dbg = nc.dram_tensor(
            "qk_dbg", [2, 2, 128, S], dt.bfloat16, kind="ExternalOutput"
        ).ap()
        vp_dbg = nc.dram_tensor(
            "vp_dbg", [128, HLOC, S // 128, 80], dt.float8e4, kind="ExternalOutput"
        ).ap()
        ain_dbg = nc.dram_tensor(
            "ain_dbg", [2, NCORES, 2 * DK, SLICE], dt.float8e4, kind="ExternalOutput"
        ).ap()
    if "h" in phases:
        av_dbg = nc.dram_tensor(
            "av_dbg", [2, DK + 1, 512], dt.float32, kind="ExternalOutput"
        ).ap()
        e2_dbg = nc.dram_tensor(
            "e2_dbg", [8, 128, 2, 2, 512], dt.float8e4, kind="ExternalOutput"
        ).ap()

    ST = S // 128  # 16 seq tiles
    FT = D // 128  # 8 feature tiles
    QC = S // 512  # 4 q-chunks for attention
    RT = B * SLICE // 128  # 4 row tiles of the output slice
    VP = 80  # padded V row stride (DoubleRow needs 16B-aligned steps)

    with tile.TileContext(nc) as tc:
        with (
            tc.tile_pool(name="singles", bufs=1) as singles,
            tc.tile_pool(name="persist", bufs=1) as persist,
            tc.tile_pool(name="dram", bufs=1, space="DRAM") as dram,
        ):
            eps_t = singles.tile([128, 1], dt.float32)
            nc.vector.memset(eps_t, EPS)
            ident = singles.tile([128, 128], dt.bfloat16)
            make_identity(nc, ident)

            # ---- persistent intermediates ----
            # x_norm^T in fp8: [128 part(d%128), chunk, f(d//128), 512 seq]
            xnt8_t = persist.tile([128, QC, FT, 512], dt.float8e4, tag="xnt8", name="xnt8")
            qT = [
                persist.tile([128, S], dt.bfloat16, tag=f"qT{m}", name=f"qT{m}")
                for m in range(2)
            ]
            kT = [
                persist.tile([128, S], dt.bfloat16, tag=f"kT{m}", name=f"kT{m}")
                for m in range(2)
            ]
            # V^T... actually V rows: [128 keys, h, kt, dk(+ones, pad 80)]
            vp8_t = persist.tile([128, HLOC, ST, VP], dt.float8e4, tag="vp8", name="vp8")
            nc.gpsimd.memset(vp8_t[:, :, :, DK : DK + 1], 1.0)

            # collective bounce buffers (fp8), one pair per head pair
            a2a_in = [
                dram.tile([NCORES, 2 * DK, SLICE], dt.float8e4, name=f"a2a_in{m}", tag=f"a2a_in{m}")
                for m in range(2)
            ]
            a2a_out = [
                dram.tile([NCORES, 2 * DK, SLICE], dt.float8e4, name=f"a2a_out{m}", tag=f"a2a_out{m}")
                for m in range(2)
            ]

            # weights
            wq_sb = singles.tile([128, FT, DLOC], dt.float8e4)
            wk_sb = singles.tile([128, FT, DLOC], dt.float8e4)
            wv_sb = singles.tile([128, FT, DLOC], dt.float8e4)
            b_bc = singles.tile([128, D], dt.float32)
            wo_sb = singles.tile([128, FT, D], dt.float8e4)
            xrb = singles.tile([128, RT, D], dt.float32)

            # ===== Phases A-C, software-pipelined =========================
            x_rows = x_b.rearrange("(t p) d -> t p d", p=128)
            with (
                tc.tile_pool(name="ln", bufs=8) as ln_pool,
                tc.tile_pool(name="lnst", bufs=12) as lnst,
                tc.tile_pool(name="epool", bufs=2) as epool,
                tc.tile_pool(name="aopool", bufs=4) as aopool,
                tc.tile_pool(name="ivpool", bufs=4) as ivpool,
            ):

                def emit_ln_chunk(c):
                    """Stage-batched LN for the chunk's 4 seq tiles: each
                    engine sees 4 independent items per stage, hiding the
                    cross-engine dependency latency."""
                    sts = list(range(4 * c, 4 * c + 4))
                    xts, mvs, rinvs, negmurs, xns, stgs = {}, {}, {}, {}, {}, {}
                    for st in sts:
                        x_t = ln_pool.tile([128, D], dt.float32, tag="x", name="x_t")
                        nc.sync.dma_start(out=x_t, in_=x_rows[st])
                        xts[st] = x_t
                    for st in sts:
                        stats = lnst.tile([128, 2, 6], dt.float32, tag="stats", name="stats")
                        for g in range(2):
                            nc.vector.bn_stats(
                                out=stats[:, g, :],
                                in_=xts[st][:, g * 512 : (g + 1) * 512],
                            )
                        mv = lnst.tile([128, 2], dt.float32, tag="mv", name="mv")
                        nc.vector.bn_aggr(out=mv, in_=stats)
                        mvs[st] = mv
                    sds = {}
                    for st in sts:
                        sd = lnst.tile([128, 1], dt.float32, tag="sd", name="sd")
                        nc.scalar.activation(
                            out=sd, in_=mvs[st][:, 1:2], func=AF.Sqrt,
                            bias=eps_t, scale=1.0,
                        )
                        sds[st] = sd
                    for st in sts:
                        rinv = lnst.tile([128, 1], dt.float32, tag="rinv", name="rinv")
                        nc.vector.reciprocal_approx_fast(out=rinv, in_=sds[st])
                        rinvs[st] = rinv
                    for st in sts:
                        negmur = lnst.tile([128, 1], dt.float32, tag="negmur", name="negmur")
                        nc.vector.tensor_scalar(
                            out=negmur,
                            in0=mvs[st][:, 0:1],
                            scalar1=rinvs[st],
                            scalar2=-1.0,
                            op0=OP.mult,
                            op1=OP.mult,
                        )
                        negmurs[st] = negmur
                    for st in sts:
                        xn = ln_pool.tile([128, D], dt.bfloat16, tag="xn", name="xn")
                        nc.scalar.activation(
                            out=xn, in_=xts[st], func=AF.Identity,
                            bias=negmurs[st], scale=rinvs[st],
                        )
                        xns[st] = xn
                    # PE transposes (per 128x128 f-tile) with the fp8 cast
                    # folded into the PSUM eviction
                    for st in sts:
                        st4 = st % 4
                        for fp in range(FT // 2):
                            tr_ps = ps_tr.tile([128, 256], dt.bfloat16, tag="tr", name="tr")
                            for k in range(2):
                                nc.tensor.transpose(
                                    tr_ps[:, k * 128 : (k + 1) * 128],
                                    xns[st][:, (2 * fp + k) * 128 : (2 * fp + k + 1) * 128],
                                    ident,
                                )
                            dst = xnt8_t[:, c, 2 * fp : 2 * fp + 2, st4 * 128 : (st4 + 1) * 128]
                            src = tr_ps.rearrange("p (k q) -> p k q", k=2)
                            if fp % 2 == 0:
                                nc.scalar.copy(out=dst, in_=src)
                            else:
                                nc.vector.tensor_copy(out=dst, in_=src)

                def emit_kq(w_sb, dst, mt, ch, evict_eng):
                    ps = ps_proj.tile([128, 512], dt.float32, tag="qkv", name="kq_ps")
                    for fp in range(FT // 2):
                        nc.tensor.matmul(
                            ps,
                            lhsT=w_sb[:, 2 * fp : 2 * fp + 2, mt * 128 : (mt + 1) * 128],
                            rhs=xnt8_t[:, ch, 2 * fp : 2 * fp + 2, :],
                            start=(fp == 0),
                            stop=(fp == FT // 2 - 1),
                            perf_mode=DR,
                        )
                    if evict_eng == "act":
                        nc.scalar.copy(
                            out=dst[mt][:, ch * 512 : (ch + 1) * 512], in_=ps
                        )
                    else:
                        nc.vector.tensor_copy(
                            out=dst[mt][:, ch * 512 : (ch + 1) * 512], in_=ps
                        )

                def emit_v(st):
                    c, st4 = st // 4, st % 4
                    ps = ps_proj.tile([128, DLOC], dt.float32, tag="qkv", name="v_ps")
                    for fp in range(FT // 2):
                        nc.tensor.matmul(
                            ps,
                            lhsT=xnt8_t[
                                :, c, 2 * fp : 2 * fp + 2, st4 * 128 : (st4 + 1) * 128
                            ],
                            rhs=wv_sb[:, 2 * fp : 2 * fp + 2, :],
                            start=(fp == 0),
                            stop=(fp == FT // 2 - 1),
                            perf_mode=DR,
                        )
                    nc.scalar.copy(
                        out=vp8_t[:, :, st, 0:DK],
                        in_=ps.rearrange("p (h d) -> p h d", h=HLOC),
                    )

                # -- block loop: LN -> K/Q (chunk c) -> V ---------------
                ps_proj_cm = tc.tile_pool(name="ps_proj", bufs=2, space="PSUM")
                ps_proj = ps_proj_cm.__enter__()
                ps_tr_cm = tc.tile_pool(name="ps_tr", bufs=2, space="PSUM")
                ps_tr = ps_tr_cm.__enter__()
                emit_ln_chunk(0)
                nc.sync.dma_start(
                    out=wk_sb, in_=wk8.rearrange("(t p) m -> p t m", p=128)
                )
                nc.sync.dma_start(
                    out=wq_sb, in_=wq8.rearrange("(t p) m -> p t m", p=128)
                )
                nc.sync.dma_start(
                    out=wv_sb, in_=wv8.rearrange("(t p) m -> p t m", p=128)
                )
                for c in range(4):
                    # 1-deep pipeline: next chunk's LN chain advances while
                    # this chunk's projections run on the PE
                    if c + 1 < 4:
                        emit_ln_chunk(c + 1)
                    for mt in range(2):
                        emit_kq(wk_sb, kT, mt, c, "act")
                    for mt in range(2):
                        emit_kq(wq_sb, qT, mt, c, "dve")
                    for st in range(4 * c, 4 * c + 4):
                        emit_v(st)
                    if c == 1:
                        nc.sync.dma_start(
                            out=b_bc,
                            in_=bass.AP(
                                tensor=b_o.tensor,
                                offset=b_o.offset,
                                ap=[[0, 128]] + list(b_o.ap),
                            ),
                        )
                        nc.sync.dma_start(
                            out=wo_sb, in_=wo8.rearrange("(t p) m -> p t m", p=128)
                        )
                        nc.sync.dma_start(
                            out=xrb,
                            in_=x_res.rearrange("b (t p) d -> p (b t) d", p=128),
                        )
                        for t in range(RT):
                            nc.vector.tensor_add(
                                out=xrb[:, t, :], in0=xrb[:, t, :], in1=b_bc
                            )
                ps_tr_cm.__exit__(None, None, None)
                ps_proj_cm.__exit__(None, None, None)

                # -- attention, head-pair-major + split AllToAll --------
                ps_s_cm = tc.tile_pool(name="ps_s", bufs=3, space="PSUM")
                ps_s = ps_s_cm.__enter__()
                ps_av_cm = tc.tile_pool(name="ps_av", bufs=1, space="PSUM")
                ps_av = ps_av_cm.__enter__()
                for hp in range(2):
                    for qc in range(QC):
                        av = [
                            ps_av.tile(
                                [DK + 1, 512],
                                dt.float32,
                                tag=f"av{j}",
                                name=f"av{hp}{j}",
                            )
                            for j in range(2)
                        ]
                        # AV trails scores/exp by one kt-pair so the PE never
                        # head-of-line blocks on an unfinished exp
                        e2s = {}
                        for m in range(ST // 2 + 1):
                            if m < ST // 2:
                                e2 = epool.tile(
                                    [128, 2, 2, 512], dt.float8e4, tag="e2", name="e2"
                                )
                                e2s[m] = e2
                                for par in range(2):
                                    kt = 2 * m + par
                                    s_ps = ps_s.tile(
                                        [128, 1024], dt.float32, tag="s", name="s_ps"
                                    )
                                    for j in range(2):
                                        nc.tensor.matmul(
                                            s_ps[:, j * 512 : (j + 1) * 512],
                                            lhsT=kT[hp][
                                                j * 64 : (j + 1) * 64,
                                                kt * 128 : (kt + 1) * 128,
                                            ],
                                            rhs=qT[hp][
                                                j * 64 : (j + 1) * 64,
                                                qc * 512 : (qc + 1) * 512,
                                            ],
                                            start=True,
                                            stop=True,
                                        )
                                    if kt % 8 < 5:
                                        # exact exp on ScalarE, fp8 out
                                        nc.scalar.activation(
                                            out=e2[:, par, :, :],
                                            in_=s_ps,
                                            func=AF.Exp,
                                            scale=float(C_EXP),
                                        )
                                    else:
                                        # exp2 bit-trick on VectorE: fp8e4 bits
                                        # = round(s*A8 + B8), saturating at 0
                                        nc.vector.tensor_scalar(
                                            out=e2[:, par, :, :].bitcast(dt.uint8),
                                            in0=s_ps,
                                            scalar1=float(A8),
                                            scalar2=float(B8),
                                            op0=OP.mult,
                                            op1=OP.add,
                                        )
                                if "h" in phases and hp == 0 and qc == 0:
                                    nc.sync.dma_start(out=e2_dbg[m], in_=e2)
                            if m >= 1:
                                mm = m - 1
                                for j in range(2):
                                    nc.tensor.matmul(
                                        av[j],
                                        lhsT=vp8_t[
                                            :, 2 * hp + j, 2 * mm : 2 * mm + 2, 0 : DK + 1
                                        ],
                                        rhs=e2s[mm][:, :, j, :],
                                        start=(mm == 0),
                                        stop=(mm == ST // 2 - 1),
                                        perf_mode=DR,
                                    )
                        if "h" in phases and hp == 0 and qc == 0:
                            for j in range(2):
                                avc = aopool.tile(
                                    [DK + 1, 512], dt.float32, tag="avc", name="avc"
                                )
                                nc.vector.tensor_copy(out=avc, in_=av[j])
                                nc.sync.dma_start(out=av_dbg[j], in_=avc)
                        # normalize + evict (DVE recip+mult from PSUM,
                        # gpsimd broadcast; gpsimd/DMA can't read PSUM)
                        for j in range(2):
                            # den must be copied to a partition-0 SBUF tile:
                            # recip straight off the partition-64 PSUM row
                            # silently reads partition 0
                            den = ivpool.tile([1, 512], dt.float32, tag="den", name="den")
                            nc.vector.tensor_copy(out=den, in_=av[j][DK : DK + 1, :])
                            invd = ivpool.tile([1, 512], dt.float32, tag="invd", name="invd")
                            nc.vector.reciprocal_approx_fast(out=invd, in_=den)
                            ibc = ivpool.tile([DK, 512], dt.float32, tag="ibc", name="ibc")
                            nc.gpsimd.partition_broadcast(ibc, invd)
                            ao = aopool.tile([DK, 512], dt.float8e4, tag="ao", name="ao")
                            nc.vector.tensor_tensor(
                                out=ao, in0=av[j][0:DK, :], in1=ibc, op=OP.mult
                            )
                            for half in range(2):
                                nc.sync.dma_start(
                                    out=a2a_in[hp][
                                        2 * qc + half, j * DK : (j + 1) * DK, :
                                    ],
                                    in_=ao[:, half * 256 : (half + 1) * 256],
                                )
                    if "g" in phases:
                        nc.sync.dma_start(out=ain_dbg[hp], in_=a2a_in[hp])
                    if "D" in phases:
                        nc.gpsimd.collective_compute(
                            "AllToAll",
                            mybir.AluOpType.bypass,
                            replica_groups=[list(range(NCORES))],
                            ins=[a2a_in[hp].opt()],
                            outs=[a2a_out[hp].opt()],
                        )

                ps_av_cm.__exit__(None, None, None)
                ps_s_cm.__exit__(None, None, None)
                if "g" in phases:
                    nc.sync.dma_start(out=xnt_dbg, in_=xnt8_t)
                    for m in range(2):
                        nc.sync.dma_start(out=qk_dbg[0, m], in_=qT[m])
                        nc.sync.dma_start(out=qk_dbg[1, m], in_=kT[m])
                    nc.sync.dma_start(out=vp_dbg, in_=vp8_t)

            # ============ Phase E: output projection ======================
            # gathered slot r of a2a_out[hp] = heads {4i+2hp, 4i+2hp+1} of
            # group i = r%4, batch r//4 -> orig f-tile 2*(r%4)+hp; wo_sb is
            # host-permuted hp-major: slot v = 4*hp + i4
            if "E" in phases:
                with (
                    tc.tile_pool(name="ps_wo", bufs=1, space="PSUM") as ps_wo,
                    tc.tile_pool(name="attg", bufs=1) as attg_pool,
                    tc.tile_pool(name="outp", bufs=4) as outp,
                ):
                    wo_ps = {}
                    for b in range(B):
                        for mt in range(SLICE // 128):
                            for oc in range(2):
                                wo_ps[b, mt, oc] = ps_wo.tile(
                                    [128, 512],
                                    dt.float32,
                                    tag=f"wo{b}{mt}{oc}",
                                    name=f"wo{b}{mt}{oc}",
                                )
                    attg = {}
                    for hp in range(2):
                        for b in range(B):
                            ag = attg_pool.tile(
                                [128, 4, SLICE],
                                dt.float8e4,
                                tag=f"ag{hp}{b}",
                                name=f"ag{hp}{b}",
                            )
                            attg[hp, b] = ag
                            nc.sync.dma_start(
                                out=ag,
                                in_=a2a_out[hp][4 * b : 4 * (b + 1), :, :].rearrange(
                                    "s (t p) q -> p (s t) q", p=128
                                ),
                            )
                        for b in range(B):
                            for mt in range(SLICE // 128):
                                for oc in range(2):
                                    for u in range(2):
                                        nc.tensor.matmul(
                                            wo_ps[b, mt, oc],
                                            lhsT=attg[hp, b][
                                                :, 2 * u : 2 * u + 2,
                                                mt * 128 : (mt + 1) * 128,
                                            ],
                                            rhs=wo_sb[
                                                :,
                                                4 * hp + 2 * u : 4 * hp + 2 * u + 2,
                                                oc * 512 : (oc + 1) * 512,
                                            ],
                                            start=(hp == 0 and u == 0),
                                            stop=(hp == 1 and u == 1),
                                            perf_mode=DR,
                                        )
                    for b in range(B):
                        for mt in range(SLICE // 128):
                            for oc in range(2):
                                o_t = outp.tile([128, 512], dt.float32, tag="o")
                                # out = psum/1024 + (residual + bias)
                                nc.vector.scalar_tensor_tensor(
                                    out=o_t,
                                    in0=wo_ps[b, mt, oc],
                                    scalar=float(1.0 / (WS * WS)),
                                    in1=xrb[
                                        :,
                                        b * (SLICE // 128) + mt,
                                        oc * 512 : (oc + 1) * 512,
                                    ],
                                    op0=OP.mult,
                                    op1=OP.add,
                                )
                                nc.sync.dma_start(
                                    out=out_sl[
                                        b,
                                        mt * 128 : (mt + 1) * 128,
                                        oc * 512 : (oc + 1) * 512,
                                    ],
                                    in_=o_t,
                                )
            else:
                nc.sync.dma_start(out=out_sl[:, :, :], in_=x_res[:, :, :])

    nc.compile()
    return nc


def _get_nc(phases="ABCDE"):
    key = ("nc", phases)
    if key not in _CACHE:
        _CACHE[key] = _build(phases)
    return _CACHE[key]


def _make_in_maps(inputs):
    x = np.asarray(inputs["x"], np.float32)
    w_q = np.asarray(inputs["w_q"], np.float32)
    w_k = np.asarray(inputs["w_k"], np.float32)
    w_v = np.asarray(inputs["w_v"], np.float32)
    w_o = np.asarray(inputs["w_o"], np.float32)
    b_o = np.asarray(inputs["b_o"], np.float32)
    gamma = np.asarray(inputs["ln_gamma"], np.float32)
    beta = np.asarray(inputs["ln_beta"], np.float32)

    assert np.allclose(beta, 0.0), "nonzero ln_beta not supported"
    # hp-major f-tile permutation for DoubleRow pairing in phase E
    woT = np.ascontiguousarray(w_o.T) * WS
    woT_p = np.concatenate(
        [woT[128 * (2 * (v % 4) + v // 4) : 128 * (2 * (v % 4) + v // 4) + 128] for v in range(8)],
        axis=0,
    ).astype(F8)
    # LN gamma folds exactly into the input side of the QKV projections
    w_qg = w_q * gamma[None, :] * WS
    w_kg = w_k * gamma[None, :] * WS
    w_vg = w_v * gamma[None, :] * WS
    in_maps = []
    for r in range(NCORES):
        b, i = r // 4, r % 4
        sl = slice(DLOC * i, DLOC * (i + 1))
        in_maps.append(
            {
                "x_b": np.ascontiguousarray(x[b]),
                "wq8": np.ascontiguousarray(w_qg[sl].T).astype(F8),
                "wk8": np.ascontiguousarray(w_kg[sl].T).astype(F8),
                "wv8": np.ascontiguousarray(w_vg[sl].T).astype(F8),
                "wo8": woT_p,
                "x_res": np.ascontiguousarray(x[:, SLICE * r : SLICE * (r + 1), :]),
                "b_o": b_o,
            }
        )
    return in_maps


def _install_ntff_hook():
    """The agent image's antenv lacks axon_hooks; recreate it so
    trace=True can capture NTFF profiles through libaxon_pjrt.so."""
    import types

    from concourse import bass_utils

    if "antenv.axon_hooks" not in sys.modules:
        import antenv
        from trn_agent_boot.trn_boot import _ntff_profile_via_ctypes

        mod = types.ModuleType("antenv.axon_hooks")
        state = {}
        mod.set_axon_ntff_profile_hook = lambda h: state.update(h=h)
        mod.get_axon_ntff_profile_hook = lambda: state.get("h")
        sys.modules["antenv.axon_hooks"] = mod
        antenv.axon_hooks = mod
        mod.set_axon_ntff_profile_hook(
            _ntff_profile_via_ctypes("/opt/axon/libaxon_pjrt.so")
        )
    bass_utils.upload_artifacts = lambda tmpdir: tmpdir


def run(inputs, trace=False, phases="ABCDE", tmpdir=None, trace_cores=None):
    from concourse import bass_utils

    if trace:
        _install_ntff_hook()
    nc = _get_nc(phases)
    in_maps = _make_in_maps(inputs)
    res = bass_utils.run_bass_kernel_spmd(
        nc,
        in_maps,
        core_ids=list(range(NCORES)),
        trace=trace,
        tmpdir=tmpdir,
        trace_cores=trace_cores,
    )
    out = np.empty((B, S, D), np.float32)
    for r in range(NCORES):
        out[:, SLICE * r : SLICE * (r + 1), :] = res.results[r]["out_sl"]
    return out, res


def kernel(**inputs):
    out, _ = run(inputs)
    return out


# revision 35
# speedup vs baseline: 1.0133x; 1.0133x over previous
"""Multi-head attention (pre-LN + residual) on 8 trn2 NeuronCores.

Sharding: core r = (batch b = r//4, head group i = r%4, 4 heads each).

Per core: LN(x_b) -> x_norm^T via DMA-crossbar transpose (fp8 copy on
gpsimd) -> fp8 DoubleRow K/Q/V projections (weights pre-scaled x32 on
host; Q/K kept x32 in bf16, V x32 in fp8) -> scores^T = K Q^T per head
(bf16, 64-contraction) -> exp split between ScalarE (exact, fp8 out)
and VectorE (exp2 bit-trick writing fp8e4 bit patterns via uint8
round+saturate) -> AV in fp8 DoubleRow over kt-pairs with a ones
column giving the softmax denominator -> normalize on gpsimd -> fp8
AllToAll per head pair -> fp8 DoubleRow w_o matmul (+1/1024 rescale
fused with bias+residual add) for a 256-row seq slice of both batches.
"""

import sys

sys.path.insert(0, "/opt/trn_rl_repo")

import numpy as np
import ml_dtypes

BF16 = ml_dtypes.bfloat16
F8 = ml_dtypes.float8_e4m3fn

# Problem constants (hardcoded per contract)
B = 2
S = 2048
D = 1024
H = 16
DK = 64
NCORES = 8
HLOC = 4  # heads per core
DLOC = HLOC * DK  # 256
SLICE = S // NCORES  # 256 output rows per batch per core
EPS = 1e-5
WS = 32.0  # fp8 weight scale (avoids e4m3 subnormals)
# scores psum = (32Q)(32K) = 1024*QK ; softmax wants exp(QK/8)
C_EXP = 1.0 / (8.0 * WS * WS)
LOG2E = 1.4426950408889634
CORR = -0.045
A8 = C_EXP * LOG2E * 8.0
B8 = (7.0 + CORR) * 8.0

_CACHE = {}


def _build(phases="ABCDE"):
    import concourse.bass as bass
    import concourse.mybir as mybir
    import concourse.tile as tile
    from concourse import bacc

    from concourse.masks import make_identity

    dt = mybir.dt
    AF = mybir.ActivationFunctionType
    OP = mybir.AluOpType
    DR = mybir.MatmulPerfMode.DoubleRow

    nc = bacc.Bacc(
        "TRN2",
        target_bir_lowering=False,
        debug=False,
        enable_asserts=False,
        num_devices=NCORES,
    )

    # ---- I/O ----
    x_b = nc.dram_tensor("x_b", [S, D], dt.float32, kind="ExternalInput").ap()
    wq8 = nc.dram_tensor("wq8", [D, DLOC], dt.float8e4, kind="ExternalInput").ap()
    wk8 = nc.dram_tensor("wk8", [D, DLOC], dt.float8e4, kind="ExternalInput").ap()
    wv8 = nc.dram_tensor("wv8", [D, DLOC], dt.float8e4, kind="ExternalInput").ap()
    wo8 = nc.dram_tensor("wo8", [D, D], dt.float8e4, kind="ExternalInput").ap()
    x_res = nc.dram_tensor(
        "x_res", [B, SLICE, D], dt.float32, kind="ExternalInput"
    ).ap()
    b_o = nc.dram_tensor("b_o", [D], dt.float32, kind="ExternalInput").ap()
    out_sl = nc.dram_tensor(
        "out_sl", [B, SLICE, D], dt.float32, kind="ExternalOutput"
    ).ap()
    if "g" in phases:
        xnt_dbg = nc.dram_tensor(
            "xnt_dbg", [128, 4, 8, 512], dt.float8e4, kind="ExternalOutput"
        ).ap()
        qk_dbg = nc.dram_tensor(
            "qk_dbg", [2, 2, 128, S], dt.bfloat16, kind="ExternalOutput"
        ).ap()
        vp_dbg = nc.dram_tensor(
            "vp_dbg", [128, HLOC, S // 128, 80], dt.float8e4, kind="ExternalOutput"
        ).ap()
        ain_dbg = nc.dram_tensor(
            "ain_dbg", [2, NCORES, 2 * DK, SLICE], dt.float8e4, kind="ExternalOutput"
        ).ap()
    if "h" in phases:
        av_dbg = nc.dram_tensor(
            "av_dbg", [2, DK + 1, 512], dt.float32, kind="ExternalOutput"
        ).ap()
        e2_dbg = nc.dram_tensor(
            "e2_dbg", [8, 128, 2, 2, 512], dt.float8e4, kind="ExternalOutput"
        ).ap()

    ST = S // 128  # 16 seq tiles
    FT = D // 128  # 8 feature tiles
    QC = S // 512  # 4 q-chunks for attention
    RT = B * SLICE // 128  # 4 row tiles of the output slice
    VP = 80  # padded V row stride (DoubleRow needs 16B-aligned steps)

    with tile.TileContext(nc) as tc:
        with (
            tc.tile_pool(name="singles", bufs=1) as singles,
            tc.tile_pool(name="persist", bufs=1) as persist,
            tc.tile_pool(name="dram", bufs=1, space="DRAM") as dram,
        ):
            eps_t = singles.tile([128, 1], dt.float32)
            nc.vector.memset(eps_t, EPS)
            ident = singles.tile([128, 128], dt.bfloat16)
            make_identity(nc, ident)

            # ---- persistent intermediates ----
            # x_norm^T in fp8: [128 part(d%128), chunk, f(d//128), 512 seq]
            xnt8_t = persist.tile([128, QC, FT, 512], dt.float8e4, tag="xnt8", name="xnt8")
            qT = [
                persist.tile([128, S], dt.bfloat16, tag=f"qT{m}", name=f"qT{m}")
                for m in range(2)
            ]
            kT = [
                persist.tile([128, S], dt.bfloat16, tag=f"kT{m}", name=f"kT{m}")
                for m in range(2)
            ]
            # V^T... actually V rows: [128 keys, h, kt, dk(+ones, pad 80)]
            vp8_t = persist.tile([128, HLOC, ST, VP], dt.float8e4, tag="vp8", name="vp8")
            nc.gpsimd.memset(vp8_t[:, :, :, DK : DK + 1], 1.0)

            # collective bounce buffers (fp8), one pair per head pair
            a2a_in = [
                dram.tile([NCORES, 2 * DK, SLICE], dt.float8e4, name=f"a2a_in{m}", tag=f"a2a_in{m}")
                for m in range(2)
            ]
            a2a_out = [
                dram.tile([NCORES, 2 * DK, SLICE], dt.float8e4, name=f"a2a_out{m}", tag=f"a2a_out{m}")
                for m in range(2)
            ]

            # weights
            wq_sb = singles.tile([128, FT, DLOC], dt.float8e4)
            wk_sb = singles.tile([128, FT, DLOC], dt.float8e4)
            wv_sb = singles.tile([128, FT, DLOC], dt.float8e4)
            b_bc = singles.tile([128, D], dt.float32)
            wo_sb = singles.tile([128, FT, D], dt.float8e4)
            xrb = singles.tile([128, RT, D], dt.float32)

            # ===== Phases A-C, software-pipelined =========================
            x_rows = x_b.rearrange("(t p) d -> t p d", p=128)
            with (
                tc.tile_pool(name="ln", bufs=8) as ln_pool,
                tc.tile_pool(name="lnst", bufs=12) as lnst,
                tc.tile_pool(name="epool", bufs=2) as epool,
                tc.tile_pool(name="aopool", bufs=4) as aopool,
                tc.tile_pool(name="ivpool", bufs=4) as ivpool,
            ):

                def emit_ln_chunk(c):
                    """Stage-batched LN for the chunk's 4 seq tiles: each
                    engine sees 4 independent items per stage, hiding the
                    cross-engine dependency latency."""
                    sts = list(range(4 * c, 4 * c + 4))
                    xts, mvs, rinvs, negmurs, xns, stgs = {}, {}, {}, {}, {}, {}
                    for st in sts:
                        x_t = ln_pool.tile([128, D], dt.float32, tag="x", name="x_t")
                        nc.sync.dma_start(out=x_t, in_=x_rows[st])
                        xts[st] = x_t
                    for st in sts:
                        stats = lnst.tile([128, 2, 6], dt.float32, tag="stats", name="stats")
                        for g in range(2):
                            nc.vector.bn_stats(
                                out=stats[:, g, :],
                                in_=xts[st][:, g * 512 : (g + 1) * 512],
                            )
                        mv = lnst.tile([128, 2], dt.float32, tag="mv", name="mv")
                        nc.vector.bn_aggr(out=mv, in_=stats)
                        mvs[st] = mv
                    sds = {}
                    for st in sts:
                        sd = lnst.tile([128, 1], dt.float32, tag="sd", name="sd")
                        nc.scalar.activation(
                            out=sd, in_=mvs[st][:, 1:2], func=AF.Sqrt,
                            bias=eps_t, scale=1.0,
                        )
                        sds[st] = sd
                    for st in sts:
                        rinv = lnst.tile([128, 1], dt.float32, tag="rinv", name="rinv")
                        nc.vector.reciprocal_approx_fast(out=rinv, in_=sds[st])
                        rinvs[st] = rinv
                    for st in sts:
                        negmur = lnst.tile([128, 1], dt.float32, tag="negmur", name="negmur")
                        nc.vector.tensor_scalar(
                            out=negmur,
                            in0=mvs[st][:, 0:1],
                            scalar1=rinvs[st],
                            scalar2=-1.0,
                            op0=OP.mult,
                            op1=OP.mult,
                        )
                        negmurs[st] = negmur
                    for st in sts:
                        xn = ln_pool.tile([128, D], dt.bfloat16, tag="xn", name="xn")
                        nc.scalar.activation(
                            out=xn, in_=xts[st], func=AF.Identity,
                            bias=negmurs[st], scale=rinvs[st],
                        )
                        xns[st] = xn
                    # PE transposes (per 128x128 f-tile) with the fp8 cast
                    # folded into the PSUM eviction
                    for st in sts:
                        st4 = st % 4
                        for fp in range(FT // 2):
                            tr_ps = ps_tr.tile([128, 256], dt.bfloat16, tag="tr", name="tr")
                            for k in range(2):
                                nc.tensor.transpose(
                                    tr_ps[:, k * 128 : (k + 1) * 128],
                                    xns[st][:, (2 * fp + k) * 128 : (2 * fp + k + 1) * 128],
                                    ident,
                                )
                            dst = xnt8_t[:, c, 2 * fp : 2 * fp + 2, st4 * 128 : (st4 + 1) * 128]
                            src = tr_ps.rearrange("p (k q) -> p k q", k=2)
                            if fp % 2 == 0:
                                nc.scalar.copy(out=dst, in_=src)
                            else:
                                nc.vector.tensor_copy(out=dst, in_=src)

                def emit_kq(w_sb, dst, mt, ch, evict_eng):
                    ps = ps_proj.tile([128, 512], dt.float32, tag="qkv", name="kq_ps")
                    for fp in range(FT // 2):
                        nc.tensor.matmul(
                            ps,
                            lhsT=w_sb[:, 2 * fp : 2 * fp + 2, mt * 128 : (mt + 1) * 128],
                            rhs=xnt8_t[:, ch, 2 * fp : 2 * fp + 2, :],
                            start=(fp == 0),
                            stop=(fp == FT // 2 - 1),
                            perf_mode=DR,
                        )
                    if evict_eng == "act":
                        nc.scalar.copy(
                            out=dst[mt][:, ch * 512 : (ch + 1) * 512], in_=ps
                        )
                    else:
                        nc.vector.tensor_copy(
                            out=dst[mt][:, ch * 512 : (ch + 1) * 512], in_=ps
                        )

                def emit_v(st):
                    c, st4 = st // 4, st % 4
                    ps = ps_proj.tile([128, DLOC], dt.float32, tag="qkv", name="v_ps")
                    for fp in range(FT // 2):
                        nc.tensor.matmul(
                            ps,
                            lhsT=xnt8_t[
                                :, c, 2 * fp : 2 * fp + 2, st4 * 128 : (st4 + 1) * 128
                            ],
                            rhs=wv_sb[:, 2 * fp : 2 * fp + 2, :],
                            start=(fp == 0),
                            stop=(fp == FT // 2 - 1),
                            perf_mode=DR,
                        )
                    nc.scalar.copy(
                        out=vp8_t[:, :, st, 0:DK],
                        in_=ps.rearrange("p (h d) -> p h d", h=HLOC),
                    )

                # -- block loop: LN -> K/Q (chunk c) -> V ---------------
                ps_proj_cm = tc.tile_pool(name="ps_proj", bufs=2, space="PSUM")
                ps_proj = ps_proj_cm.__enter__()
                ps_tr_cm = tc.tile_pool(name="ps_tr", bufs=2, space="PSUM")
                ps_tr = ps_tr_cm.__enter__()
                emit_ln_chunk(0)
                nc.sync.dma_start(
                    out=wk_sb, in_=wk8.rearrange("(t p) m -> p t m", p=128)
                )
                nc.sync.dma_start(
                    out=wq_sb, in_=wq8.rearrange("(t p) m -> p t m", p=128)
                )
                nc.sync.dma_start(
                    out=wv_sb, in_=wv8.rearrange("(t p) m -> p t m", p=128)
                )
                for c in range(4):
                    # 1-deep pipeline: next chunk's LN chain advances while
                    # this chunk's projections run on the PE
                    if c + 1 < 4:
                        emit_ln_chunk(c + 1)
                    for mt in range(2):
                        emit_kq(wk_sb, kT, mt, c, "act")
                    for mt in range(2):
                        emit_kq(wq_sb, qT, mt, c, "dve")
                    for st in range(4 * c, 4 * c + 4):
                        emit_v(st)
                    if c == 1:
                        nc.sync.dma_start(
                            out=b_bc,
                            in_=bass.AP(
                                tensor=b_o.tensor,
                                offset=b_o.offset,
                                ap=[[0, 128]] + list(b_o.ap),
                            ),
                        )
                        nc.sync.dma_start(
                            out=wo_sb, in_=wo8.rearrange("(t p) m -> p t m", p=128)
                        )
                        nc.sync.dma_start(
                            out=xrb,
                            in_=x_res.rearrange("b (t p) d -> p (b t) d", p=128),
                        )
                        for t in range(RT):
                            nc.vector.tensor_add(
                                out=xrb[:, t, :], in0=xrb[:, t, :], in1=b_bc
                            )
                ps_tr_cm.__exit__(None, None, None)
                ps_proj_cm.__exit__(None, None, None)

                # -- attention, head-pair-major + split AllToAll --------
                ps_s_cm = tc.tile_pool(name="ps_s", bufs=2, space="PSUM")
                ps_s = ps_s_cm.__enter__()
                ps_av_cm = tc.tile_pool(name="ps_av", bufs=2, space="PSUM")
                ps_av = ps_av_cm.__enter__()
                for hp in range(2):
                    for qc in range(QC):
                        av = [
                            ps_av.tile(
                                [DK + 1, 512],
                                dt.float32,
                                tag=f"av{j}",
                                name=f"av{hp}{j}",
                            )
                            for j in range(2)
                        ]
                        # AV trails scores/exp by one kt-pair so the PE never
                        # head-of-line blocks on an unfinished exp
                        e2s = {}
                        for m in range(ST // 2 + 1):
                            if m < ST // 2:
                                e2 = epool.tile(
                                    [128, 2, 2, 512], dt.float8e4, tag="e2", name="e2"
                                )
                                e2s[m] = e2
                                for par in range(2):
                                    kt = 2 * m + par
                                    s_ps = ps_s.tile(
                                        [128, 1024], dt.float32, tag="s", name="s_ps"
                                    )
                                    for j in range(2):
                                        nc.tensor.matmul(
                                            s_ps[:, j * 512 : (j + 1) * 512],
                                            lhsT=kT[hp][
                                                j * 64 : (j + 1) * 64,
                                                kt * 128 : (kt + 1) * 128,
                                            ],
                                            rhs=qT[hp][
                                                j * 64 : (j + 1) * 64,
                                                qc * 512 : (qc + 1) * 512,
                                            ],
                                            start=True,
                                            stop=True,
                                        )
                                    if kt % 8 < 5:
                                        # exact exp on ScalarE, fp8 out
                                        nc.scalar.activation(
                                            out=e2[:, par, :, :],
                                            in_=s_ps,
                                            func=AF.Exp,
                                            scale=float(C_EXP),
                                        )
                                    else:
                                        # exp2 bit-trick on VectorE: fp8e4 bits
                                        # = round(s*A8 + B8), saturating at 0
                                        nc.vector.tensor_scalar(
                                            out=e2[:, par, :, :].bitcast(dt.uint8),
                                            in0=s_ps,
                                            scalar1=float(A8),
                                            scalar2=float(B8),
                                            op0=OP.mult,
                                            op1=OP.add,
                                        )
                                if "h" in phases and hp == 0 and qc == 0:
                                    nc.sync.dma_start(out=e2_dbg[m], in_=e2)
                            if m >= 1:
                                mm = m - 1
                                for j in range(2):
                                    nc.tensor.matmul(
                                        av[j],
                                        lhsT=vp8_t[
                                            :, 2 * hp + j, 2 * mm : 2 * mm + 2, 0 : DK + 1
                                        ],
                                        rhs=e2s[mm][:, :, j, :],
                                        start=(mm == 0),
                                        stop=(mm == ST // 2 - 1),
                                        perf_mode=DR,
                                    )
                        if "h" in phases and hp == 0 and qc == 0:
                            for j in range(2):
                                avc = aopool.tile(
                                    [DK + 1, 512], dt.float32, tag="avc", name="avc"
                                )
                                nc.vector.tensor_copy(out=avc, in_=av[j])
                                nc.sync.dma_start(out=av_dbg[j], in_=avc)
                        # normalize + evict (DVE recip+mult from PSUM,
                        # gpsimd broadcast; gpsimd/DMA can't read PSUM)
                        for j in range(2):
                            # den must be copied to a partition-0 SBUF tile:
                            # recip straight off the partition-64 PSUM row
                            # silently reads partition 0
                            den = ivpool.tile([1, 512], dt.float32, tag="den", name="den")
                            nc.scalar.copy(out=den, in_=av[j][DK : DK + 1, :])
                            invd = ivpool.tile([1, 512], dt.float32, tag="invd", name="invd")
                            nc.vector.reciprocal_approx_fast(out=invd, in_=den)
                            ibc = ivpool.tile([DK, 512], dt.float32, tag="ibc", name="ibc")
                            nc.gpsimd.partition_broadcast(ibc, invd)
                            ao = aopool.tile([DK, 512], dt.float8e4, tag="ao", name="ao")
                            nc.vector.tensor_tensor(
                                out=ao, in0=av[j][0:DK, :], in1=ibc, op=OP.mult
                            )
                            for half in range(2):
                                nc.sync.dma_start(
                                    out=a2a_in[hp][
                                        2 * qc + half, j * DK : (j + 1) * DK, :
                                    ],
                                    in_=ao[:, half * 256 : (half + 1) * 256],
                                )
                    if "g" in phases:
                        nc.sync.dma_start(out=ain_dbg[hp], in_=a2a_in[hp])
                    if "D" in phases:
                        nc.gpsimd.collective_compute(
                            "AllToAll",
                            mybir.AluOpType.bypass,
                            replica_groups=[list(range(NCORES))],
                            ins=[a2a_in[hp].opt()],
                            outs=[a2a_out[hp].opt()],
                        )

                ps_av_cm.__exit__(None, None, None)
                ps_s_cm.__exit__(None, None, None)
                if "g" in phases:
                    nc.sync.dma_start(out=xnt_dbg, in_=xnt8_t)
                    for m in range(2):
                        nc.sync.dma_start(out=qk_dbg[0, m], in_=qT[m])
                        nc.sync.dma_start(out=qk_dbg[1, m], in_=kT[m])
                    nc.sync.dma_start(out=vp_dbg, in_=vp8_t)

            # ============ Phase E: output projection ======================
            # gathered slot r of a2a_out[hp] = heads {4i+2hp, 4i+2hp+1} of
            # group i = r%4, batch r//4 -> orig f-tile 2*(r%4)+hp; wo_sb is
            # host-permuted hp-major: slot v = 4*hp + i4
            if "E" in phases:
                with (
                    tc.tile_pool(name="ps_wo", bufs=1, space="PSUM") as ps_wo,
                    tc.tile_pool(name="attg", bufs=1) as attg_pool,
                    tc.tile_pool(name="outp", bufs=4) as outp,
                ):
                    wo_ps = {}
                    for b in range(B):
                        for mt in range(SLICE // 128):
                            for oc in range(2):
                                wo_ps[b, mt, oc] = ps_wo.tile(
                                    [128, 512],
                                    dt.float32,
                                    tag=f"wo{b}{mt}{oc}",
                                    name=f"wo{b}{mt}{oc}",
                                )
                    attg = {}
                    for hp in range(2):
                        for b in range(B):
                            ag = attg_pool.tile(
                                [128, 4, SLICE],
                                dt.float8e4,
                                tag=f"ag{hp}{b}",
                                name=f"ag{hp}{b}",
                            )
                            attg[hp, b] = ag
                            nc.sync.dma_start(
                                out=ag,
                                in_=a2a_out[hp][4 * b : 4 * (b + 1), :, :].rearrange(
                                    "s (t p) q -> p (s t) q", p=128
                                ),
                            )
                        for b in range(B):
                            for mt in range(SLICE // 128):
                                for oc in range(2):
                                    for u in range(2):
                                        nc.tensor.matmul(
                                            wo_ps[b, mt, oc],
                                            lhsT=attg[hp, b][
                                                :, 2 * u : 2 * u + 2,
                                                mt * 128 : (mt + 1) * 128,
                                            ],
                                            rhs=wo_sb[
                                                :,
                                                4 * hp + 2 * u : 4 * hp + 2 * u + 2,
                                                oc * 512 : (oc + 1) * 512,
                                            ],
                                            start=(hp == 0 and u == 0),
                                            stop=(hp == 1 and u == 1),
                                            perf_mode=DR,
                                        )
                    for b in range(B):
                        for mt in range(SLICE // 128):
                            for oc in range(2):
                                o_t = outp.tile([128, 512], dt.float32, tag="o")
                                # out = psum/1024 + (residual + bias)
                                nc.vector.scalar_tensor_tensor(
                                    out=o_t,
                                    in0=wo_ps[b, mt, oc],
                                    scalar=float(1.0 / (WS * WS)),
                                    in1=xrb[
                                        :,
                                        b * (SLICE // 128) + mt,
                                        oc * 512 : (oc + 1) * 512,
                                    ],
                                    op0=OP.mult,
                                    op1=OP.add,
                                )
                                nc.sync.dma_start(
                                    out=out_sl[
                                        b,
                                        mt * 128 : (mt + 1) * 128,
                                        oc * 512 : (oc + 1) * 512,
                                    ],
                                    in_=o_t,
                                )
            else:
                nc.sync.dma_start(out=out_sl[:, :, :], in_=x_res[:, :, :])

    nc.compile()
    return nc


def _get_nc(phases="ABCDE"):
    key = ("nc", phases)
    if key not in _CACHE:
        _CACHE[key] = _build(phases)
    return _CACHE[key]


def _make_in_maps(inputs):
    x = np.asarray(inputs["x"], np.float32)
    w_q = np.asarray(inputs["w_q"], np.float32)
    w_k = np.asarray(inputs["w_k"], np.float32)
    w_v = np.asarray(inputs["w_v"], np.float32)
    w_o = np.asarray(inputs["w_o"], np.float32)
    b_o = np.asarray(inputs["b_o"], np.float32)
    gamma = np.asarray(inputs["ln_gamma"], np.float32)
    beta = np.asarray(inputs["ln_beta"], np.float32)

    assert np.allclose(beta, 0.0), "nonzero ln_beta not supported"
    # hp-major f-tile permutation for DoubleRow pairing in phase E
    woT = np.ascontiguousarray(w_o.T) * WS
    woT_p = np.concatenate(
        [woT[128 * (2 * (v % 4) + v // 4) : 128 * (2 * (v % 4) + v // 4) + 128] for v in range(8)],
        axis=0,
    ).astype(F8)
    # LN gamma folds exactly into the input side of the QKV projections
    w_qg = w_q * gamma[None, :] * WS
    w_kg = w_k * gamma[None, :] * WS
    w_vg = w_v * gamma[None, :] * WS
    in_maps = []
    for r in range(NCORES):
        b, i = r // 4, r % 4
        sl = slice(DLOC * i, DLOC * (i + 1))
        in_maps.append(
            {
                "x_b": np.ascontiguousarray(x[b]),
                "wq8": np.ascontiguousarray(w_qg[sl].T).astype(F8),
                "wk8": np.ascontiguousarray(w_kg[sl].T).astype(F8),
                "wv8": np.ascontiguousarray(w_vg[sl].T).astype(F8),
                "wo8": woT_p,
                "x_res": np.ascontiguousarray(x[:, SLICE * r : SLICE * (r + 1), :]),
                "b_o": b_o,
            }
        )
    return in_maps


def _install_ntff_hook():
    """The agent image's antenv lacks axon_hooks; recreate it so
    trace=True can capture NTFF profiles through libaxon_pjrt.so."""
    import types

    from concourse import bass_utils

    if "antenv.axon_hooks" not in sys.modules:
        import antenv
        from trn_agent_boot.trn_boot import _ntff_profile_via_ctypes

        mod = types.ModuleType("antenv.axon_hooks")
        state = {}
        mod.set_axon_ntff_profile_hook = lambda h: state.update(h=h)
        mod.get_axon_ntff_profile_hook = lambda: state.get("h")
        sys.modules["antenv.axon_hooks"] = mod
        antenv.axon_hooks = mod
        mod.set_axon_ntff_profile_hook(
            _ntff_profile_via_ctypes("/opt/axon/libaxon_pjrt.so")
        )
    bass_utils.upload_artifacts = lambda tmpdir: tmpdir


def run(inputs, trace=False, phases="ABCDE", tmpdir=None, trace_cores=None):
    from concourse import bass_utils

    if trace:
        _install_ntff_hook()
    nc = _get_nc(phases)
    in_maps = _make_in_maps(inputs)
    res = bass_utils.run_bass_kernel_spmd(
        nc,
        in_maps,
        core_ids=list(range(NCORES)),
        trace=trace,
        tmpdir=tmpdir,
        trace_cores=trace_cores,
    )
    out = np.empty((B, S, D), np.float32)
    for r in range(NCORES):
        out[:, SLICE * r : SLICE * (r + 1), :] = res.results[r]["out_sl"]
    return out, res


def kernel(**inputs):
    out, _ = run(inputs)
    return out


# revision 37
# speedup vs baseline: 1.1553x; 1.1401x over previous
"""Multi-head attention (pre-LN + residual) on 8 trn2 NeuronCores.

Sharding: core r = (batch b = r//4, head group i = r%4, 4 heads each).

Per core: LN(x_b) -> x_norm^T via DMA-crossbar transpose (fp8 copy on
gpsimd) -> fp8 DoubleRow K/Q/V projections (weights pre-scaled x32 on
host; Q/K kept x32 in bf16, V x32 in fp8) -> scores^T = K Q^T per head
(bf16, 64-contraction) -> exp split between ScalarE (exact, fp8 out)
and VectorE (exp2 bit-trick writing fp8e4 bit patterns via uint8
round+saturate) -> AV in fp8 DoubleRow over kt-pairs with a ones
column giving the softmax denominator -> normalize on gpsimd -> fp8
AllToAll per head pair -> fp8 DoubleRow w_o matmul (+1/1024 rescale
fused with bias+residual add) for a 256-row seq slice of both batches.
"""

import sys

sys.path.insert(0, "/opt/trn_rl_repo")

import numpy as np
import ml_dtypes

BF16 = ml_dtypes.bfloat16
F8 = ml_dtypes.float8_e4m3fn

# Problem constants (hardcoded per contract)
B = 2
S = 2048
D = 1024
H = 16
DK = 64
NCORES = 8
HLOC = 4  # heads per core
DLOC = HLOC * DK  # 256
SLICE = S // NCORES  # 256 output rows per batch per core
EPS = 1e-5
WS = 32.0  # fp8 weight scale (avoids e4m3 subnormals)
# scores psum = (32Q)(32K) = 1024*QK ; softmax wants exp(QK/8)
C_EXP = 1.0 / (8.0 * WS * WS)
LOG2E = 1.4426950408889634
CORR = -0.045
A8 = C_EXP * LOG2E * 8.0
B8 = (7.0 + CORR) * 8.0

_CACHE = {}


def _build(phases="ABCDE"):
    import concourse.bass as bass
    import concourse.mybir as mybir
    import concourse.tile as tile
    from concourse import bacc

    from concourse.masks import make_identity

    dt = mybir.dt
    AF = mybir.ActivationFunctionType
    OP = mybir.AluOpType
    DR = mybir.MatmulPerfMode.DoubleRow

    nc = bacc.Bacc(
        "TRN2",
        target_bir_lowering=False,
        debug=False,
        enable_asserts=False,
        num_devices=NCORES,
    )

    # ---- I/O ----
    x_b = nc.dram_tensor("x_b", [S, D], dt.float32, kind="ExternalInput").ap()
    wq8 = nc.dram_tensor("wq8", [D, DLOC], dt.float8e4, kind="ExternalInput").ap()
    wk8 = nc.dram_tensor("wk8", [D, DLOC], dt.float8e4, kind="ExternalInput").ap()
    wv8 = nc.dram_tensor("wv8", [D, DLOC], dt.float8e4, kind="ExternalInput").ap()
    wo8 = nc.dram_tensor("wo8", [D, D], dt.float8e4, kind="ExternalInput").ap()
    x_res = nc.dram_tensor(
        "x_res", [B, SLICE, D], dt.float32, kind="ExternalInput"
    ).ap()
    b_o = nc.dram_tensor("b_o", [D], dt.float32, kind="ExternalInput").ap()
    out_sl = nc.dram_tensor(
        "out_sl", [B, SLICE, D], dt.float32, kind="ExternalOutput"
    ).ap()
    if "g" in phases:
        xnt_dbg = nc.dram_tensor(
            "xnt_dbg", [128, 4, 8, 512], dt.float8e4, kind="ExternalOutput"
        ).ap()
        qk_dbg = nc.dram_tensor(
            "qk_dbg", [2, 2, 128, S], dt.bfloat16, kind="ExternalOutput"
        ).ap()
        vp_dbg = nc.dram_tensor(
            "vp_dbg", [128, HLOC, S // 128, 80], dt.float8e4, kind="ExternalOutput"
        ).ap()
        ain_dbg = nc.dram_tensor(
            "ain_dbg", [2, NCORES, 2 * DK, SLICE], dt.float8e4, kind="ExternalOutput"
        ).ap()
    if "h" in phases:
        av_dbg = nc.dram_tensor(
            "av_dbg", [2, DK + 1, 512], dt.float32, kind="ExternalOutput"
        ).ap()
        e2_dbg = nc.dram_tensor(
            "e2_dbg", [8, 128, 2, 2, 512], dt.float8e4, kind="ExternalOutput"
        ).ap()

    ST = S // 128  # 16 seq tiles
    FT = D // 128  # 8 feature tiles
    QC = S // 512  # 4 q-chunks for attention
    RT = B * SLICE // 128  # 4 row tiles of the output slice
    VP = 80  # padded V row stride (DoubleRow needs 16B-aligned steps)

    with tile.TileContext(nc) as tc:
        with (
            tc.tile_pool(name="singles", bufs=1) as singles,
            tc.tile_pool(name="persist", bufs=1) as persist,
            tc.tile_pool(name="dram", bufs=1, space="DRAM") as dram,
        ):
            eps_t = singles.tile([128, 1], dt.float32)
            nc.vector.memset(eps_t, EPS)
            ident = singles.tile([128, 128], dt.bfloat16)
            make_identity(nc, ident)

            # ---- persistent intermediates ----
            # x_norm^T in fp8: [128 part(d%128), chunk, f(d//128), 512 seq]
            xnt8_t = persist.tile([128, QC, FT, 512], dt.float8e4, tag="xnt8", name="xnt8")
            qT = [
                persist.tile([128, S], dt.bfloat16, tag=f"qT{m}", name=f"qT{m}")
                for m in range(2)
            ]
            kT = [
                persist.tile([128, S], dt.bfloat16, tag=f"kT{m}", name=f"kT{m}")
                for m in range(2)
            ]
            # V^T... actually V rows: [128 keys, h, kt, dk(+ones, pad 80)]
            vp8_t = persist.tile([128, HLOC, ST, VP], dt.float8e4, tag="vp8", name="vp8")
            nc.gpsimd.memset(vp8_t[:, :, :, DK : DK + 1], 1.0)

            # collective bounce buffers (fp8), one pair per head pair
            a2a_in = [
                dram.tile([NCORES, 2 * DK, SLICE], dt.float8e4, name=f"a2a_in{m}", tag=f"a2a_in{m}")
                for m in range(2)
            ]
            a2a_out = [
                dram.tile([NCORES, 2 * DK, SLICE], dt.float8e4, name=f"a2a_out{m}", tag=f"a2a_out{m}")
                for m in range(2)
            ]

            # weights
            wq_sb = singles.tile([128, FT, DLOC], dt.float8e4)
            wk_sb = singles.tile([128, FT, DLOC], dt.float8e4)
            wv_sb = singles.tile([128, FT, DLOC], dt.float8e4)
            b_bc = singles.tile([128, D], dt.float32)
            wo_sb = singles.tile([128, FT, D], dt.float8e4)
            xrb = singles.tile([128, RT, D], dt.float32)

            # ===== Phases A-C, software-pipelined =========================
            x_rows = x_b.rearrange("(t p) d -> t p d", p=128)
            with (
                tc.tile_pool(name="ln", bufs=8) as ln_pool,
                tc.tile_pool(name="lnst", bufs=12) as lnst,
                tc.tile_pool(name="epool", bufs=2) as epool,
                tc.tile_pool(name="aopool", bufs=4) as aopool,
                tc.tile_pool(name="ivpool", bufs=4) as ivpool,
            ):

                def emit_ln_chunk(c):
                    """Stage-batched LN for the chunk's 4 seq tiles: each
                    engine sees 4 independent items per stage, hiding the
                    cross-engine dependency latency."""
                    sts = list(range(4 * c, 4 * c + 4))
                    xts, mvs, rinvs, negmurs, xns, stgs = {}, {}, {}, {}, {}, {}
                    for st in sts:
                        x_t = ln_pool.tile([128, D], dt.float32, tag="x", name="x_t")
                        nc.sync.dma_start(out=x_t, in_=x_rows[st])
                        xts[st] = x_t
                    for st in sts:
                        stats = lnst.tile([128, 2, 6], dt.float32, tag="stats", name="stats")
                        for g in range(2):
                            nc.vector.bn_stats(
                                out=stats[:, g, :],
                                in_=xts[st][:, g * 512 : (g + 1) * 512],
                            )
                        mv = lnst.tile([128, 2], dt.float32, tag="mv", name="mv")
                        nc.vector.bn_aggr(out=mv, in_=stats)
                        mvs[st] = mv
                    sds = {}
                    for st in sts:
                        sd = lnst.tile([128, 1], dt.float32, tag="sd", name="sd")
                        nc.scalar.activation(
                            out=sd, in_=mvs[st][:, 1:2], func=AF.Sqrt,
                            bias=eps_t, scale=1.0,
                        )
                        sds[st] = sd
                    for st in sts:
                        rinv = lnst.tile([128, 1], dt.float32, tag="rinv", name="rinv")
                        nc.vector.reciprocal_approx_fast(out=rinv, in_=sds[st])
                        rinvs[st] = rinv
                    for st in sts:
                        negmur = lnst.tile([128, 1], dt.float32, tag="negmur", name="negmur")
                        nc.vector.tensor_scalar(
                            out=negmur,
                            in0=mvs[st][:, 0:1],
                            scalar1=rinvs[st],
                            scalar2=-1.0,
                            op0=OP.mult,
                            op1=OP.mult,
                        )
                        negmurs[st] = negmur
                    for st in sts:
                        xn = ln_pool.tile([128, D], dt.bfloat16, tag="xn", name="xn")
                        nc.scalar.activation(
                            out=xn, in_=xts[st], func=AF.Identity,
                            bias=negmurs[st], scale=rinvs[st],
                        )
                        xns[st] = xn
                    # PE transposes (per 128x128 f-tile) with the fp8 cast
                    # folded into the PSUM eviction
                    for st in sts:
                        st4 = st % 4
                        for fp in range(FT // 2):
                            tr_ps = ps_tr.tile([128, 256], dt.bfloat16, tag="tr", name="tr")
                            for k in range(2):
                                nc.tensor.transpose(
                                    tr_ps[:, k * 128 : (k + 1) * 128],
                                    xns[st][:, (2 * fp + k) * 128 : (2 * fp + k + 1) * 128],
                                    ident,
                                )
                            dst = xnt8_t[:, c, 2 * fp : 2 * fp + 2, st4 * 128 : (st4 + 1) * 128]
                            src = tr_ps.rearrange("p (k q) -> p k q", k=2)
                            if fp % 2 == 0:
                                nc.scalar.copy(out=dst, in_=src)
                            else:
                                nc.vector.tensor_copy(out=dst, in_=src)

                def emit_kq(w_sb, dst, mt, ch, evict_eng):
                    ps = ps_proj.tile([128, 512], dt.float32, tag="qkv", name="kq_ps")
                    for fp in range(FT // 2):
                        nc.tensor.matmul(
                            ps,
                            lhsT=w_sb[:, 2 * fp : 2 * fp + 2, mt * 128 : (mt + 1) * 128],
                            rhs=xnt8_t[:, ch, 2 * fp : 2 * fp + 2, :],
                            start=(fp == 0),
                            stop=(fp == FT // 2 - 1),
                            perf_mode=DR,
                        )
                    if evict_eng == "act":
                        nc.scalar.copy(
                            out=dst[mt][:, ch * 512 : (ch + 1) * 512], in_=ps
                        )
                    else:
                        nc.vector.tensor_copy(
                            out=dst[mt][:, ch * 512 : (ch + 1) * 512], in_=ps
                        )

                def emit_v(st):
                    c, st4 = st // 4, st % 4
                    ps = ps_proj.tile([128, DLOC], dt.float32, tag="qkv", name="v_ps")
                    for fp in range(FT // 2):
                        nc.tensor.matmul(
                            ps,
                            lhsT=xnt8_t[
                                :, c, 2 * fp : 2 * fp + 2, st4 * 128 : (st4 + 1) * 128
                            ],
                            rhs=wv_sb[:, 2 * fp : 2 * fp + 2, :],
                            start=(fp == 0),
                            stop=(fp == FT // 2 - 1),
                            perf_mode=DR,
                        )
                    nc.scalar.copy(
                        out=vp8_t[:, :, st, 0:DK],
                        in_=ps.rearrange("p (h d) -> p h d", h=HLOC),
                    )

                # -- block loop: LN -> K/Q (chunk c) -> V ---------------
                ps_proj_cm = tc.tile_pool(name="ps_proj", bufs=2, space="PSUM")
                ps_proj = ps_proj_cm.__enter__()
                ps_tr_cm = tc.tile_pool(name="ps_tr", bufs=2, space="PSUM")
                ps_tr = ps_tr_cm.__enter__()
                emit_ln_chunk(0)
                nc.sync.dma_start(
                    out=wk_sb, in_=wk8.rearrange("(t p) m -> p t m", p=128)
                )
                nc.sync.dma_start(
                    out=wq_sb, in_=wq8.rearrange("(t p) m -> p t m", p=128)
                )
                nc.sync.dma_start(
                    out=wv_sb, in_=wv8.rearrange("(t p) m -> p t m", p=128)
                )
                for c in range(4):
                    # 1-deep pipeline: next chunk's LN chain advances while
                    # this chunk's projections run on the PE
                    if c + 1 < 4:
                        emit_ln_chunk(c + 1)
                    for mt in range(2):
                        emit_kq(wk_sb, kT, mt, c, "act")
                    for mt in range(2):
                        emit_kq(wq_sb, qT, mt, c, "dve")
                    for st in range(4 * c, 4 * c + 4):
                        emit_v(st)
                    if c == 1:
                        nc.sync.dma_start(
                            out=b_bc,
                            in_=bass.AP(
                                tensor=b_o.tensor,
                                offset=b_o.offset,
                                ap=[[0, 128]] + list(b_o.ap),
                            ),
                        )
                        nc.sync.dma_start(
                            out=wo_sb, in_=wo8.rearrange("(t p) m -> p t m", p=128)
                        )
                        nc.sync.dma_start(
                            out=xrb,
                            in_=x_res.rearrange("b (t p) d -> p (b t) d", p=128),
                        )
                        for t in range(RT):
                            nc.vector.tensor_add(
                                out=xrb[:, t, :], in0=xrb[:, t, :], in1=b_bc
                            )
                ps_tr_cm.__exit__(None, None, None)
                ps_proj_cm.__exit__(None, None, None)

                # -- attention, head-pair-major + split AllToAll --------
                ps_s_cm = tc.tile_pool(name="ps_s", bufs=3, space="PSUM")
                ps_s = ps_s_cm.__enter__()
                ps_av_cm = tc.tile_pool(name="ps_av", bufs=1, space="PSUM")
                ps_av = ps_av_cm.__enter__()
                for hp in range(2):
                    for qc in range(QC):
                        av = [
                            ps_av.tile(
                                [DK + 1, 512],
                                dt.float32,
                                tag=f"av{j}",
                                name=f"av{hp}{j}",
                            )
                            for j in range(2)
                        ]
                        # AV trails scores/exp by one kt-pair so the PE never
                        # head-of-line blocks on an unfinished exp
                        e2s = {}
                        for m in range(ST // 2 + 1):
                            if m < ST // 2:
                                e2 = epool.tile(
                                    [128, 2, 2, 512], dt.float8e4, tag="e2", name="e2"
                                )
                                e2s[m] = e2
                                for par in range(2):
                                    kt = 2 * m + par
                                    s_ps = ps_s.tile(
                                        [128, 1024], dt.float32, tag="s", name="s_ps"
                                    )
                                    for j in range(2):
                                        nc.tensor.matmul(
                                            s_ps[:, j * 512 : (j + 1) * 512],
                                            lhsT=kT[hp][
                                                j * 64 : (j + 1) * 64,
                                                kt * 128 : (kt + 1) * 128,
                                            ],
                                            rhs=qT[hp][
                                                j * 64 : (j + 1) * 64,
                                                qc * 512 : (qc + 1) * 512,
                                            ],
                                            start=True,
                                            stop=True,
                                        )
                                    if kt % 8 < 5:
                                        # exact exp on ScalarE, fp8 out
                                        nc.scalar.activation(
                                            out=e2[:, par, :, :],
                                            in_=s_ps,
                                            func=AF.Exp,
                                            scale=float(C_EXP),
                                        )
                                    else:
                                        # exp2 bit-trick on VectorE: fp8e4 bits
                                        # = round(s*A8 + B8), saturating at 0
                                        nc.vector.tensor_scalar(
                                            out=e2[:, par, :, :].bitcast(dt.uint8),
                                            in0=s_ps,
                                            scalar1=float(A8),
                                            scalar2=float(B8),
                                            op0=OP.mult,
                                            op1=OP.add,
                                        )
                                if "h" in phases and hp == 0 and qc == 0:
                                    nc.sync.dma_start(out=e2_dbg[m], in_=e2)
                            if m >= 1:
                                mm = m - 1
                                for j in range(2):
                                    nc.tensor.matmul(
                                        av[j],
                                        lhsT=vp8_t[
                                            :, 2 * hp + j, 2 * mm : 2 * mm + 2, 0 : DK + 1
                                        ],
                                        rhs=e2s[mm][:, :, j, :],
                                        start=(mm == 0),
                                        stop=(mm == ST // 2 - 1),
                                        perf_mode=DR,
                                    )
                        if "h" in phases and hp == 0 and qc == 0:
                            for j in range(2):
                                avc = aopool.tile(
                                    [DK + 1, 512], dt.float32, tag="avc", name="avc"
                                )
                                nc.vector.tensor_copy(out=avc, in_=av[j])
                                nc.sync.dma_start(out=av_dbg[j], in_=avc)
                        # normalize + evict (DVE recip+mult from PSUM,
                        # gpsimd broadcast; gpsimd/DMA can't read PSUM)
                        for j in range(2):
                            # den must be copied to a partition-0 SBUF tile:
                            # recip straight off the partition-64 PSUM row
                            # silently reads partition 0
                            den = ivpool.tile([1, 512], dt.float32, tag="den", name="den")
                            nc.vector.tensor_copy(out=den, in_=av[j][DK : DK + 1, :])
                            invd = ivpool.tile([1, 512], dt.float32, tag="invd", name="invd")
                            nc.vector.reciprocal_approx_fast(out=invd, in_=den)
                            ibc = ivpool.tile([DK, 512], dt.float32, tag="ibc", name="ibc")
                            nc.gpsimd.partition_broadcast(ibc, invd)
                            ao = aopool.tile([DK, 512], dt.float8e4, tag="ao", name="ao")
                            nc.vector.tensor_tensor(
                                out=ao, in0=av[j][0:DK, :], in1=ibc, op=OP.mult
                            )
                            for half in range(2):
                                nc.sync.dma_start(
                                    out=a2a_in[hp][
                                        2 * qc + half, j * DK : (j + 1) * DK, :
                                    ],
                                    in_=ao[:, half * 256 : (half + 1) * 256],
                                )
                    if "g" in phases:
                        nc.sync.dma_start(out=ain_dbg[hp], in_=a2a_in[hp])
                    if "D" in phases:
                        nc.gpsimd.collective_compute(
                            "AllToAll",
                            mybir.AluOpType.bypass,
                            replica_groups=[list(range(NCORES))],
                            ins=[a2a_in[hp].opt()],
                            outs=[a2a_out[hp].opt()],
                        )

                ps_av_cm.__exit__(None, None, None)
                ps_s_cm.__exit__(None, None, None)
                if "g" in phases:
                    nc.sync.dma_start(out=xnt_dbg, in_=xnt8_t)
                    for m in range(2):
                        nc.sync.dma_start(out=qk_dbg[0, m], in_=qT[m])
                        nc.sync.dma_start(out=qk_dbg[1, m], in_=kT[m])
                    nc.sync.dma_start(out=vp_dbg, in_=vp8_t)

            # ============ Phase E: output projection ======================
            # gathered slot r of a2a_out[hp] = heads {4i+2hp, 4i+2hp+1} of
            # group i = r%4, batch r//4 -> orig f-tile 2*(r%4)+hp; wo_sb is
            # host-permuted hp-major: slot v = 4*hp + i4
            if "E" in phases:
                with (
                    tc.tile_pool(name="ps_wo", bufs=1, space="PSUM") as ps_wo,
                    tc.tile_pool(name="attg", bufs=1) as attg_pool,
                    tc.tile_pool(name="outp", bufs=4) as outp,
                ):
                    wo_ps = {}
                    for b in range(B):
                        for mt in range(SLICE // 128):
                            for oc in range(2):
                                wo_ps[b, mt, oc] = ps_wo.tile(
                                    [128, 512],
                                    dt.float32,
                                    tag=f"wo{b}{mt}{oc}",
                                    name=f"wo{b}{mt}{oc}",
                                )
                    attg = {}
                    for hp in range(2):
                        for b in range(B):
                            ag = attg_pool.tile(
                                [128, 4, SLICE],
                                dt.float8e4,
                                tag=f"ag{hp}{b}",
                                name=f"ag{hp}{b}",
                            )
                            attg[hp, b] = ag
                            nc.sync.dma_start(
                                out=ag,
                                in_=a2a_out[hp][4 * b : 4 * (b + 1), :, :].rearrange(
                                    "s (t p) q -> p (s t) q", p=128
                                ),
                            )
                        for b in range(B):
                            for mt in range(SLICE // 128):
                                for oc in range(2):
                                    for u in range(2):
                                        nc.tensor.matmul(
                                            wo_ps[b, mt, oc],
                                            lhsT=attg[hp, b][
                                                :, 2 * u : 2 * u + 2,
                                                mt * 128 : (mt + 1) * 128,
                                            ],
                                            rhs=wo_sb[
                                                :,
                                                4 * hp + 2 * u : 4 * hp + 2 * u + 2,
                                                oc * 512 : (oc + 1) * 512,
                                            ],
                                            start=(hp == 0 and u == 0),
                                            stop=(hp == 1 and u == 1),
                                            perf_mode=DR,
                                        )
                    for b in range(B):
                        for mt in range(SLICE // 128):
                            for oc in range(2):
                                o_t = outp.tile([128, 512], dt.float32, tag="o")
                                # out = psum/1024 + (residual + bias)
                                nc.vector.scalar_tensor_tensor(
                                    out=o_t,
                                    in0=wo_ps[b, mt, oc],
                                    scalar=float(1.0 / (WS * WS)),
                                    in1=xrb[
                                        :,
                                        b * (SLICE // 128) + mt,
                                        oc * 512 : (oc + 1) * 512,
                                    ],
                                    op0=OP.mult,
                                    op1=OP.add,
                                )
                                nc.sync.dma_start(
                                    out=out_sl[
                                        b,
                                        mt * 128 : (mt + 1) * 128,
                                        oc * 512 : (oc + 1) * 512,
                                    ],
                                    in_=o_t,
                                )
            else:
                nc.sync.dma_start(out=out_sl[:, :, :], in_=x_res[:, :, :])

    nc.compile()
    return nc


def _get_nc(phases="ABCDE"):
    key = ("nc", phases)
    if key not in _CACHE:
        _CACHE[key] = _build(phases)
    return _CACHE[key]


def _make_in_maps(inputs):
    x = np.asarray(inputs["x"], np.float32)
    w_q = np.asarray(inputs["w_q"], np.float32)
    w_k = np.asarray(inputs["w_k"], np.float32)
    w_v = np.asarray(inputs["w_v"], np.float32)
    w_o = np.asarray(inputs["w_o"], np.float32)
    b_o = np.asarray(inputs["b_o"], np.float32)
    gamma = np.asarray(inputs["ln_gamma"], np.float32)
    beta = np.asarray(inputs["ln_beta"], np.float32)

    assert np.allclose(beta, 0.0), "nonzero ln_beta not supported"
    # hp-major f-tile permutation for DoubleRow pairing in phase E
    woT = np.ascontiguousarray(w_o.T) * WS
    woT_p = np.concatenate(
        [woT[128 * (2 * (v % 4) + v // 4) : 128 * (2 * (v % 4) + v // 4) + 128] for v in range(8)],
        axis=0,
    ).astype(F8)
    # LN gamma folds exactly into the input side of the QKV projections
    w_qg = w_q * gamma[None, :] * WS
    w_kg = w_k * gamma[None, :] * WS
    w_vg = w_v * gamma[None, :] * WS
    in_maps = []
    for r in range(NCORES):
        b, i = r // 4, r % 4
        sl = slice(DLOC * i, DLOC * (i + 1))
        in_maps.append(
            {
                "x_b": np.ascontiguousarray(x[b]),
                "wq8": np.ascontiguousarray(w_qg[sl].T).astype(F8),
                "wk8": np.ascontiguousarray(w_kg[sl].T).astype(F8),
                "wv8": np.ascontiguousarray(w_vg[sl].T).astype(F8),
                "wo8": woT_p,
                "x_res": np.ascontiguousarray(x[:, SLICE * r : SLICE * (r + 1), :]),
                "b_o": b_o,
            }
        )
    return in_maps


def _install_ntff_hook():
    """The agent image's antenv lacks axon_hooks; recreate it so
    trace=True can capture NTFF profiles through libaxon_pjrt.so."""
    import types

    from concourse import bass_utils

    if "antenv.axon_hooks" not in sys.modules:
        import antenv
        from trn_agent_boot.trn_boot import _ntff_profile_via_ctypes

        mod = types.ModuleType("antenv.axon_hooks")
        state = {}
        mod.set_axon_ntff_profile_hook = lambda h: state.update(h=h)
        mod.get_axon_ntff_profile_hook = lambda: state.get("h")
        sys.modules["antenv.axon_hooks"] = mod
        antenv.axon_hooks = mod
        mod.set_axon_ntff_profile_hook(
            _ntff_profile_via_ctypes("/opt/axon/libaxon_pjrt.so")
        )
    bass_utils.upload_artifacts = lambda tmpdir: tmpdir


def run(inputs, trace=False, phases="ABCDE", tmpdir=None, trace_cores=None):
    from concourse import bass_utils

    if trace:
        _install_ntff_hook()
    nc = _get_nc(phases)
    in_maps = _make_in_maps(inputs)
    res = bass_utils.run_bass_kernel_spmd(
        nc,
        in_maps,
        core_ids=list(range(NCORES)),
        trace=trace,
        tmpdir=tmpdir,
        trace_cores=trace_cores,
    )
    out = np.empty((B, S, D), np.float32)
    for r in range(NCORES):
        out[:, SLICE * r : SLICE * (r + 1), :] = res.results[r]["out_sl"]
    return out, res


def kernel(**inputs):
    out, _ = run(inputs)
    return out


# revision 46
# speedup vs baseline: 1.2065x; 1.0444x over previous
"""Multi-head attention (pre-LN + residual) on 8 trn2 NeuronCores.

Sharding: core r = (batch b = r//4, head group i = r%4, 4 heads each).

Per core: stage-batched LN over 4-seq-tile chunks (1-deep chunk
pipeline) -> PE transpose of x_norm with the fp8e4 cast folded into
the PSUM eviction -> fp8 DoubleRow K/Q/V projections (weights
pre-scaled x32 on host to dodge e4m3 subnormals; Q/K evicted x32 in
bf16, V x32 in fp8 with a padded-80 row stride and a ones column) ->
scores^T = K Q^T per head (bf16, 64-contraction) -> exp split 10/6
between ScalarE (exact exp, fp8e4 out) and VectorE (exp2 bit-trick:
fp8e4 bit pattern = round(psum*A8+B8) via uint8 store w/ saturation;
softmax ratio cancels the shared approximation error) -> AV in fp8
DoubleRow over kt-pairs, software-pipelined to trail scores/exp by 2
kt-pairs (ones column gives the denominator) -> normalize (ACT den
copy, DVE recip+mult, gpsimd broadcast) -> fp8 AllToAll per head
pair, the first overlapped under the second pair's attention -> fp8
DoubleRow w_o matmul against hp-major-permuted w_o (+1/1024 rescale
fused with bias+residual via scalar_tensor_tensor) producing a
256-row seq slice of both batches.
"""

import sys

sys.path.insert(0, "/opt/trn_rl_repo")

import numpy as np
import ml_dtypes

BF16 = ml_dtypes.bfloat16
F8 = ml_dtypes.float8_e4m3fn

# Problem constants (hardcoded per contract)
B = 2
S = 2048
D = 1024
H = 16
DK = 64
NCORES = 8
HLOC = 4  # heads per core
DLOC = HLOC * DK  # 256
SLICE = S // NCORES  # 256 output rows per batch per core
EPS = 1e-5
WS = 32.0  # fp8 weight scale (avoids e4m3 subnormals)
# scores psum = (32Q)(32K) = 1024*QK ; softmax wants exp(QK/8)
C_EXP = 1.0 / (8.0 * WS * WS)
LOG2E = 1.4426950408889634
CORR = -0.045
A8 = C_EXP * LOG2E * 8.0
B8 = (7.0 + CORR) * 8.0

_CACHE = {}


def _build(phases="ABCDE"):
    import concourse.bass as bass
    import concourse.mybir as mybir
    import concourse.tile as tile
    from concourse import bacc

    from concourse.masks import make_identity

    dt = mybir.dt
    AF = mybir.ActivationFunctionType
    OP = mybir.AluOpType
    DR = mybir.MatmulPerfMode.DoubleRow

    nc = bacc.Bacc(
        "TRN2",
        target_bir_lowering=False,
        debug=False,
        enable_asserts=False,
        num_devices=NCORES,
    )

    # ---- I/O ----
    x_b = nc.dram_tensor("x_b", [S, D], dt.float32, kind="ExternalInput").ap()
    wq8 = nc.dram_tensor("wq8", [D, DLOC], dt.float8e4, kind="ExternalInput").ap()
    wk8 = nc.dram_tensor("wk8", [D, DLOC], dt.float8e4, kind="ExternalInput").ap()
    wv8 = nc.dram_tensor("wv8", [D, DLOC], dt.float8e4, kind="ExternalInput").ap()
    wo8 = nc.dram_tensor("wo8", [D, D], dt.float8e4, kind="ExternalInput").ap()
    x_res = nc.dram_tensor(
        "x_res", [B, SLICE, D], dt.float32, kind="ExternalInput"
    ).ap()
    b_o = nc.dram_tensor("b_o", [D], dt.float32, kind="ExternalInput").ap()
    out_sl = nc.dram_tensor(
        "out_sl", [B, SLICE, D], dt.float32, kind="ExternalOutput"
    ).ap()
    if "g" in phases:
        xnt_dbg = nc.dram_tensor(
            "xnt_dbg", [128, 4, 8, 512], dt.float8e4, kind="ExternalOutput"
        ).ap()
        qk_dbg = nc.dram_tensor(
            "qk_dbg", [2, 2, 128, S], dt.bfloat16, kind="ExternalOutput"
        ).ap()
        vp_dbg = nc.dram_tensor(
            "vp_dbg", [128, HLOC, S // 128, 80], dt.float8e4, kind="ExternalOutput"
        ).ap()
        ain_dbg = nc.dram_tensor(
            "ain_dbg", [2, NCORES, 2 * DK, SLICE], dt.float8e4, kind="ExternalOutput"
        ).ap()
    if "h" in phases:
        av_dbg = nc.dram_tensor(
            "av_dbg", [2, DK + 1, 512], dt.float32, kind="ExternalOutput"
        ).ap()
        e2_dbg = nc.dram_tensor(
            "e2_dbg", [8, 128, 2, 2, 512], dt.float8e4, kind="ExternalOutput"
        ).ap()

    ST = S // 128  # 16 seq tiles
    FT = D // 128  # 8 feature tiles
    QC = S // 512  # 4 q-chunks for attention
    RT = B * SLICE // 128  # 4 row tiles of the output slice
    VP = 80  # padded V row stride (DoubleRow needs 16B-aligned steps)

    with tile.TileContext(nc) as tc:
        with (
            tc.tile_pool(name="singles", bufs=1) as singles,
            tc.tile_pool(name="persist", bufs=1) as persist,
            tc.tile_pool(name="dram", bufs=1, space="DRAM") as dram,
        ):
            eps_t = singles.tile([128, 1], dt.float32)
            nc.vector.memset(eps_t, EPS)
            ident = singles.tile([128, 128], dt.bfloat16)
            make_identity(nc, ident)

            # ---- persistent intermediates ----
            # x_norm^T in fp8: [128 part(d%128), chunk, f(d//128), 512 seq]
            xnt8_t = persist.tile([128, QC, FT, 512], dt.float8e4, tag="xnt8", name="xnt8")
            qT = [
                persist.tile([128, S], dt.bfloat16, tag=f"qT{m}", name=f"qT{m}")
                for m in range(2)
            ]
            kT = [
                persist.tile([128, S], dt.bfloat16, tag=f"kT{m}", name=f"kT{m}")
                for m in range(2)
            ]
            # V^T... actually V rows: [128 keys, h, kt, dk(+ones, pad 80)]
            vp8_t = persist.tile([128, HLOC, ST, VP], dt.float8e4, tag="vp8", name="vp8")
            nc.gpsimd.memset(vp8_t[:, :, :, DK : DK + 1], 1.0)

            # collective bounce buffers (fp8), one pair per head pair
            a2a_in = [
                dram.tile([NCORES, 2 * DK, SLICE], dt.float8e4, name=f"a2a_in{m}", tag=f"a2a_in{m}")
                for m in range(2)
            ]
            a2a_out = [
                dram.tile([NCORES, 2 * DK, SLICE], dt.float8e4, name=f"a2a_out{m}", tag=f"a2a_out{m}")
                for m in range(2)
            ]

            # weights
            wq_sb = singles.tile([128, FT, DLOC], dt.float8e4)
            wk_sb = singles.tile([128, FT, DLOC], dt.float8e4)
            wv_sb = singles.tile([128, FT, DLOC], dt.float8e4)
            b_bc = singles.tile([128, D], dt.float32)
            wo_sb = singles.tile([128, FT, D], dt.float8e4)
            xrb = singles.tile([128, RT, D], dt.float32)

            # ===== Phases A-C, software-pipelined =========================
            x_rows = x_b.rearrange("(t p) d -> t p d", p=128)
            with (
                tc.tile_pool(name="ln", bufs=8) as ln_pool,
                tc.tile_pool(name="lnst", bufs=12) as lnst,
                tc.tile_pool(name="epool", bufs=6) as epool,
                tc.tile_pool(name="aopool", bufs=4) as aopool,
                tc.tile_pool(name="ivpool", bufs=4) as ivpool,
            ):

                def emit_ln_chunk(c):
                    """Stage-batched LN for the chunk's 4 seq tiles: each
                    engine sees 4 independent items per stage, hiding the
                    cross-engine dependency latency."""
                    sts = list(range(4 * c, 4 * c + 4))
                    xts, mvs, rinvs, negmurs, xns, stgs = {}, {}, {}, {}, {}, {}
                    for st in sts:
                        x_t = ln_pool.tile([128, D], dt.float32, tag="x", name="x_t")
                        nc.sync.dma_start(out=x_t, in_=x_rows[st])
                        xts[st] = x_t
                    for st in sts:
                        stats = lnst.tile([128, 2, 6], dt.float32, tag="stats", name="stats")
                        for g in range(2):
                            nc.vector.bn_stats(
                                out=stats[:, g, :],
                                in_=xts[st][:, g * 512 : (g + 1) * 512],
                            )
                        mv = lnst.tile([128, 2], dt.float32, tag="mv", name="mv")
                        nc.vector.bn_aggr(out=mv, in_=stats)
                        mvs[st] = mv
                    sds = {}
                    for st in sts:
                        sd = lnst.tile([128, 1], dt.float32, tag="sd", name="sd")
                        nc.scalar.activation(
                            out=sd, in_=mvs[st][:, 1:2], func=AF.Sqrt,
                            bias=eps_t, scale=1.0,
                        )
                        sds[st] = sd
                    for st in sts:
                        rinv = lnst.tile([128, 1], dt.float32, tag="rinv", name="rinv")
                        nc.vector.reciprocal_approx_fast(out=rinv, in_=sds[st])
                        rinvs[st] = rinv
                    for st in sts:
                        negmur = lnst.tile([128, 1], dt.float32, tag="negmur", name="negmur")
                        nc.vector.tensor_scalar(
                            out=negmur,
                            in0=mvs[st][:, 0:1],
                            scalar1=rinvs[st],
                            scalar2=-1.0,
                            op0=OP.mult,
                            op1=OP.mult,
                        )
                        negmurs[st] = negmur
                    for st in sts:
                        xn = ln_pool.tile([128, D], dt.bfloat16, tag="xn", name="xn")
                        nc.scalar.activation(
                            out=xn, in_=xts[st], func=AF.Identity,
                            bias=negmurs[st], scale=rinvs[st],
                        )
                        xns[st] = xn
                    # PE transposes (per 128x128 f-tile) with the fp8 cast
                    # folded into the PSUM eviction
                    for st in sts:
                        st4 = st % 4
                        for fp in range(FT // 2):
                            tr_ps = ps_tr.tile([128, 256], dt.bfloat16, tag="tr", name="tr")
                            for k in range(2):
                                nc.tensor.transpose(
                                    tr_ps[:, k * 128 : (k + 1) * 128],
                                    xns[st][:, (2 * fp + k) * 128 : (2 * fp + k + 1) * 128],
                                    ident,
                                )
                            dst = xnt8_t[:, c, 2 * fp : 2 * fp + 2, st4 * 128 : (st4 + 1) * 128]
                            src = tr_ps.rearrange("p (k q) -> p k q", k=2)
                            if fp % 2 == 0:
                                nc.scalar.copy(out=dst, in_=src)
                            else:
                                nc.vector.tensor_copy(out=dst, in_=src)

                def emit_kq(w_sb, dst, mt, ch, evict_eng):
                    ps = ps_proj.tile([128, 512], dt.float32, tag="qkv", name="kq_ps")
                    for fp in range(FT // 2):
                        nc.tensor.matmul(
                            ps,
                            lhsT=w_sb[:, 2 * fp : 2 * fp + 2, mt * 128 : (mt + 1) * 128],
                            rhs=xnt8_t[:, ch, 2 * fp : 2 * fp + 2, :],
                            start=(fp == 0),
                            stop=(fp == FT // 2 - 1),
                            perf_mode=DR,
                        )
                    if evict_eng == "act":
                        nc.scalar.copy(
                            out=dst[mt][:, ch * 512 : (ch + 1) * 512], in_=ps
                        )
                    else:
                        nc.vector.tensor_copy(
                            out=dst[mt][:, ch * 512 : (ch + 1) * 512], in_=ps
                        )

                def emit_v(st):
                    c, st4 = st // 4, st % 4
                    ps = ps_proj.tile([128, DLOC], dt.float32, tag="qkv", name="v_ps")
                    for fp in range(FT // 2):
                        nc.tensor.matmul(
                            ps,
                            lhsT=xnt8_t[
                                :, c, 2 * fp : 2 * fp + 2, st4 * 128 : (st4 + 1) * 128
                            ],
                            rhs=wv_sb[:, 2 * fp : 2 * fp + 2, :],
                            start=(fp == 0),
                            stop=(fp == FT // 2 - 1),
                            perf_mode=DR,
                        )
                    nc.scalar.copy(
                        out=vp8_t[:, :, st, 0:DK],
                        in_=ps.rearrange("p (h d) -> p h d", h=HLOC),
                    )

                # -- block loop: LN -> K/Q (chunk c) -> V ---------------
                ps_proj_cm = tc.tile_pool(name="ps_proj", bufs=2, space="PSUM")
                ps_proj = ps_proj_cm.__enter__()
                ps_tr_cm = tc.tile_pool(name="ps_tr", bufs=2, space="PSUM")
                ps_tr = ps_tr_cm.__enter__()
                emit_ln_chunk(0)
                nc.sync.dma_start(
                    out=wk_sb, in_=wk8.rearrange("(t p) m -> p t m", p=128)
                )
                nc.sync.dma_start(
                    out=wq_sb, in_=wq8.rearrange("(t p) m -> p t m", p=128)
                )
                nc.sync.dma_start(
                    out=wv_sb, in_=wv8.rearrange("(t p) m -> p t m", p=128)
                )
                for c in range(4):
                    # 1-deep pipeline: next chunk's LN chain advances while
                    # this chunk's projections run on the PE
                    if c + 1 < 4:
                        emit_ln_chunk(c + 1)
                    for mt in range(2):
                        emit_kq(wk_sb, kT, mt, c, "act")
                    for mt in range(2):
                        emit_kq(wq_sb, qT, mt, c, "dve")
                    for st in range(4 * c, 4 * c + 4):
                        emit_v(st)
                    if c == 1:
                        nc.sync.dma_start(
                            out=b_bc,
                            in_=bass.AP(
                                tensor=b_o.tensor,
                                offset=b_o.offset,
                                ap=[[0, 128]] + list(b_o.ap),
                            ),
                        )
                        nc.sync.dma_start(
                            out=wo_sb, in_=wo8.rearrange("(t p) m -> p t m", p=128)
                        )
                        nc.sync.dma_start(
                            out=xrb,
                            in_=x_res.rearrange("b (t p) d -> p (b t) d", p=128),
                        )
                        for t in range(RT):
                            nc.vector.tensor_add(
                                out=xrb[:, t, :], in0=xrb[:, t, :], in1=b_bc
                            )
                ps_tr_cm.__exit__(None, None, None)
                ps_proj_cm.__exit__(None, None, None)

                # -- attention, head-pair-major + split AllToAll --------
                ps_s_cm = tc.tile_pool(name="ps_s", bufs=3, space="PSUM")
                ps_s = ps_s_cm.__enter__()
                ps_av_cm = tc.tile_pool(name="ps_av", bufs=1, space="PSUM")
                ps_av = ps_av_cm.__enter__()
                for hp in range(2):
                    for qc in range(QC):
                        av = [
                            ps_av.tile(
                                [DK + 1, 512],
                                dt.float32,
                                tag=f"av{j}",
                                name=f"av{hp}{j}",
                            )
                            for j in range(2)
                        ]
                        # AV trails scores/exp by one kt-pair so the PE never
                        # head-of-line blocks on an unfinished exp
                        e2s = {}
                        for m in range(ST // 2 + 2):
                            if m < ST // 2:
                                e2 = epool.tile(
                                    [128, 2, 2, 512], dt.float8e4, tag="e2", name="e2"
                                )
                                e2s[m] = e2
                                for par in range(2):
                                    kt = 2 * m + par
                                    s_ps = ps_s.tile(
                                        [128, 1024], dt.float32, tag="s", name="s_ps"
                                    )
                                    for j in range(2):
                                        nc.tensor.matmul(
                                            s_ps[:, j * 512 : (j + 1) * 512],
                                            lhsT=kT[hp][
                                                j * 64 : (j + 1) * 64,
                                                kt * 128 : (kt + 1) * 128,
                                            ],
                                            rhs=qT[hp][
                                                j * 64 : (j + 1) * 64,
                                                qc * 512 : (qc + 1) * 512,
                                            ],
                                            start=True,
                                            stop=True,
                                        )
                                    if kt % 8 < 5:
                                        # exact exp on ScalarE, fp8 out
                                        nc.scalar.activation(
                                            out=e2[:, par, :, :],
                                            in_=s_ps,
                                            func=AF.Exp,
                                            scale=float(C_EXP),
                                        )
                                    else:
                                        # exp2 bit-trick on VectorE: fp8e4 bits
                                        # = round(s*A8 + B8), saturating at 0
                                        nc.vector.tensor_scalar(
                                            out=e2[:, par, :, :].bitcast(dt.uint8),
                                            in0=s_ps,
                                            scalar1=float(A8),
                                            scalar2=float(B8),
                                            op0=OP.mult,
                                            op1=OP.add,
                                        )
                                if "h" in phases and hp == 0 and qc == 0:
                                    nc.sync.dma_start(out=e2_dbg[m], in_=e2)
                            if m >= 2:
                                mm = m - 2
                                for j in range(2):
                                    nc.tensor.matmul(
                                        av[j],
                                        lhsT=vp8_t[
                                            :, 2 * hp + j, 2 * mm : 2 * mm + 2, 0 : DK + 1
                                        ],
                                        rhs=e2s[mm][:, :, j, :],
                                        start=(mm == 0),
                                        stop=(mm == ST // 2 - 1),
                                        perf_mode=DR,
                                    )
                        if "h" in phases and hp == 0 and qc == 0:
                            for j in range(2):
                                avc = aopool.tile(
                                    [DK + 1, 512], dt.float32, tag="avc", name="avc"
                                )
                                nc.vector.tensor_copy(out=avc, in_=av[j])
                                nc.sync.dma_start(out=av_dbg[j], in_=avc)
                        # normalize + evict (DVE recip+mult from PSUM,
                        # gpsimd broadcast; gpsimd/DMA can't read PSUM)
                        for j in range(2):
                            # den must be copied to a partition-0 SBUF tile:
                            # recip straight off the partition-64 PSUM row
                            # silently reads partition 0
                            den = ivpool.tile([1, 512], dt.float32, tag="den", name="den")
                            nc.vector.tensor_copy(out=den, in_=av[j][DK : DK + 1, :])
                            invd = ivpool.tile([1, 512], dt.float32, tag="invd", name="invd")
                            nc.vector.reciprocal_approx_fast(out=invd, in_=den)
                            ibc = ivpool.tile([DK, 512], dt.float32, tag="ibc", name="ibc")
                            nc.gpsimd.partition_broadcast(ibc, invd)
                            ao = aopool.tile([DK, 512], dt.float8e4, tag="ao", name="ao")
                            nc.vector.tensor_tensor(
                                out=ao, in0=av[j][0:DK, :], in1=ibc, op=OP.mult
                            )
                            for half in range(2):
                                nc.sync.dma_start(
                                    out=a2a_in[hp][
                                        2 * qc + half, j * DK : (j + 1) * DK, :
                                    ],
                                    in_=ao[:, half * 256 : (half + 1) * 256],
                                )
                    if "g" in phases:
                        nc.sync.dma_start(out=ain_dbg[hp], in_=a2a_in[hp])
                    if "D" in phases:
                        nc.gpsimd.collective_compute(
                            "AllToAll",
                            mybir.AluOpType.bypass,
                            replica_groups=[list(range(NCORES))],
                            ins=[a2a_in[hp].opt()],
                            outs=[a2a_out[hp].opt()],
                        )

                ps_av_cm.__exit__(None, None, None)
                ps_s_cm.__exit__(None, None, None)
                if "g" in phases:
                    nc.sync.dma_start(out=xnt_dbg, in_=xnt8_t)
                    for m in range(2):
                        nc.sync.dma_start(out=qk_dbg[0, m], in_=qT[m])
                        nc.sync.dma_start(out=qk_dbg[1, m], in_=kT[m])
                    nc.sync.dma_start(out=vp_dbg, in_=vp8_t)

            # ============ Phase E: output projection ======================
            # gathered slot r of a2a_out[hp] = heads {4i+2hp, 4i+2hp+1} of
            # group i = r%4, batch r//4 -> orig f-tile 2*(r%4)+hp; wo_sb is
            # host-permuted hp-major: slot v = 4*hp + i4
            if "E" in phases:
                with (
                    tc.tile_pool(name="ps_wo", bufs=1, space="PSUM") as ps_wo,
                    tc.tile_pool(name="attg", bufs=1) as attg_pool,
                    tc.tile_pool(name="outp", bufs=4) as outp,
                ):
                    wo_ps = {}
                    for b in range(B):
                        for mt in range(SLICE // 128):
                            for oc in range(2):
                                wo_ps[b, mt, oc] = ps_wo.tile(
                                    [128, 512],
                                    dt.float32,
                                    tag=f"wo{b}{mt}{oc}",
                                    name=f"wo{b}{mt}{oc}",
                                )
                    attg = {}
                    for hp in range(2):
                        for b in range(B):
                            ag = attg_pool.tile(
                                [128, 4, SLICE],
                                dt.float8e4,
                                tag=f"ag{hp}{b}",
                                name=f"ag{hp}{b}",
                            )
                            attg[hp, b] = ag
                            nc.sync.dma_start(
                                out=ag,
                                in_=a2a_out[hp][4 * b : 4 * (b + 1), :, :].rearrange(
                                    "s (t p) q -> p (s t) q", p=128
                                ),
                            )
                        for b in range(B):
                            for mt in range(SLICE // 128):
                                for oc in range(2):
                                    for u in range(2):
                                        nc.tensor.matmul(
                                            wo_ps[b, mt, oc],
                                            lhsT=attg[hp, b][
                                                :, 2 * u : 2 * u + 2,
                                                mt * 128 : (mt + 1) * 128,
                                            ],
                                            rhs=wo_sb[
                                                :,
                                                4 * hp + 2 * u : 4 * hp + 2 * u + 2,
                                                oc * 512 : (oc + 1) * 512,
                                            ],
                                            start=(hp == 0 and u == 0),
                                            stop=(hp == 1 and u == 1),
                                            perf_mode=DR,
                                        )
                    for b in range(B):
                        for mt in range(SLICE // 128):
                            for oc in range(2):
                                o_t = outp.tile([128, 512], dt.float32, tag="o")
                                # out = psum/1024 + (residual + bias)
                                nc.vector.scalar_tensor_tensor(
                                    out=o_t,
                                    in0=wo_ps[b, mt, oc],
                                    scalar=float(1.0 / (WS * WS)),
                                    in1=xrb[
                                        :,
                                        b * (SLICE // 128) + mt,
                                        oc * 512 : (oc + 1) * 512,
                                    ],
                                    op0=OP.mult,
                                    op1=OP.add,
                                )
                                nc.sync.dma_start(
                                    out=out_sl[
                                        b,
                                        mt * 128 : (mt + 1) * 128,
                                        oc * 512 : (oc + 1) * 512,
                                    ],
                                    in_=o_t,
                                )
            else:
                nc.sync.dma_start(out=out_sl[:, :, :], in_=x_res[:, :, :])

    nc.compile()
    return nc


def _get_nc(phases="ABCDE"):
    key = ("nc", phases)
    if key not in _CACHE:
        _CACHE[key] = _build(phases)
    return _CACHE[key]


def _make_in_maps(inputs):
    x = np.asarray(inputs["x"], np.float32)
    w_q = np.asarray(inputs["w_q"], np.float32)
    w_k = np.asarray(inputs["w_k"], np.float32)
    w_v = np.asarray(inputs["w_v"], np.float32)
    w_o = np.asarray(inputs["w_o"], np.float32)
    b_o = np.asarray(inputs["b_o"], np.float32)
    gamma = np.asarray(inputs["ln_gamma"], np.float32)
    beta = np.asarray(inputs["ln_beta"], np.float32)

    assert np.allclose(beta, 0.0), "nonzero ln_beta not supported"
    # hp-major f-tile permutation for DoubleRow pairing in phase E
    woT = np.ascontiguousarray(w_o.T) * WS
    woT_p = np.concatenate(
        [woT[128 * (2 * (v % 4) + v // 4) : 128 * (2 * (v % 4) + v // 4) + 128] for v in range(8)],
        axis=0,
    ).astype(F8)
    # LN gamma folds exactly into the input side of the QKV projections
    w_qg = w_q * gamma[None, :] * WS
    w_kg = w_k * gamma[None, :] * WS
    w_vg = w_v * gamma[None, :] * WS
    in_maps = []
    for r in range(NCORES):
        b, i = r // 4, r % 4
        sl = slice(DLOC * i, DLOC * (i + 1))
        in_maps.append(
            {
                "x_b": np.ascontiguousarray(x[b]),
                "wq8": np.ascontiguousarray(w_qg[sl].T).astype(F8),
                "wk8": np.ascontiguousarray(w_kg[sl].T).astype(F8),
                "wv8": np.ascontiguousarray(w_vg[sl].T).astype(F8),
                "wo8": woT_p,
                "x_res": np.ascontiguousarray(x[:, SLICE * r : SLICE * (r + 1), :]),
                "b_o": b_o,
            }
        )
    return in_maps


def _install_ntff_hook():
    """The agent image's antenv lacks axon_hooks; recreate it so
    trace=True can capture NTFF profiles through libaxon_pjrt.so."""
    import types

    from concourse import bass_utils

    if "antenv.axon_hooks" not in sys.modules:
        import antenv
        from trn_agent_boot.trn_boot import _ntff_profile_via_ctypes

        mod = types.ModuleType("antenv.axon_hooks")
        state = {}
        mod.set_axon_ntff_profile_hook = lambda h: state.update(h=h)
        mod.get_axon_ntff_profile_hook = lambda: state.get("h")
        sys.modules["antenv.axon_hooks"] = mod
        antenv.axon_hooks = mod
        mod.set_axon_ntff_profile_hook(
            _ntff_profile_via_ctypes("/opt/axon/libaxon_pjrt.so")
        )
    bass_utils.upload_artifacts = lambda tmpdir: tmpdir


def run(inputs, trace=False, phases="ABCDE", tmpdir=None, trace_cores=None):
    from concourse import bass_utils

    if trace:
        _install_ntff_hook()
    nc = _get_nc(phases)
    in_maps = _make_in_maps(inputs)
    res = bass_utils.run_bass_kernel_spmd(
        nc,
        in_maps,
        core_ids=list(range(NCORES)),
        trace=trace,
        tmpdir=tmpdir,
        trace_cores=trace_cores,
    )
    out = np.empty((B, S, D), np.float32)
    for r in range(NCORES):
        out[:, SLICE * r : SLICE * (r + 1), :] = res.results[r]["out_sl"]
    return out, res


def kernel(**inputs):
    out, _ = run(inputs)
    return out


# revision 50
# speedup vs baseline: 1.2751x; 1.0569x over previous
"""Multi-head attention (pre-LN + residual) on 8 trn2 NeuronCores.

Sharding: core r = (batch b = r//4, head group i = r%4, 4 heads each).

Per core: stage-batched LN over 4-seq-tile chunks (1-deep chunk
pipeline) -> PE transpose of x_norm with the fp8e4 cast folded into
the PSUM eviction -> fp8 DoubleRow K/Q/V projections (weights
pre-scaled x32 on host to dodge e4m3 subnormals; Q/K evicted x32 in
bf16, V x32 in fp8 with a padded-80 row stride and a ones column) ->
scores^T = K Q^T per head (bf16, 64-contraction) -> exp split 10/6
between ScalarE (exact exp, fp8e4 out) and VectorE (exp2 bit-trick:
fp8e4 bit pattern = round(psum*A8+B8) via uint8 store w/ saturation;
softmax ratio cancels the shared approximation error) -> AV in fp8
DoubleRow over kt-pairs, software-pipelined to trail scores/exp by 2
kt-pairs (ones column gives the denominator) -> normalize (ACT den
copy, DVE recip+mult, gpsimd broadcast) -> fp8 AllToAll per head
pair, the first overlapped under the second pair's attention -> fp8
DoubleRow w_o matmul against hp-major-permuted w_o (+1/1024 rescale
fused with bias+residual via scalar_tensor_tensor) producing a
256-row seq slice of both batches.
"""

import sys

sys.path.insert(0, "/opt/trn_rl_repo")

import numpy as np
import ml_dtypes

BF16 = ml_dtypes.bfloat16
F8 = ml_dtypes.float8_e4m3fn

# Problem constants (hardcoded per contract)
B = 2
S = 2048
D = 1024
H = 16
DK = 64
NCORES = 8
HLOC = 4  # heads per core
DLOC = HLOC * DK  # 256
SLICE = S // NCORES  # 256 output rows per batch per core
EPS = 1e-5
WS = 32.0  # fp8 weight scale (avoids e4m3 subnormals)
# scores psum = (32Q)(32K) = 1024*QK ; softmax wants exp(QK/8)
C_EXP = 1.0 / (8.0 * WS * WS)
LOG2E = 1.4426950408889634
CORR = -0.045
A8 = C_EXP * LOG2E * 8.0
B8 = (7.0 + CORR) * 8.0

_CACHE = {}


def _build(phases="ABCDE"):
    import concourse.bass as bass
    import concourse.mybir as mybir
    import concourse.tile as tile
    from concourse import bacc

    from concourse.masks import make_identity

    dt = mybir.dt
    AF = mybir.ActivationFunctionType
    OP = mybir.AluOpType
    DR = mybir.MatmulPerfMode.DoubleRow

    nc = bacc.Bacc(
        "TRN2",
        target_bir_lowering=False,
        debug=False,
        enable_asserts=False,
        num_devices=NCORES,
    )

    # ---- I/O ----
    x_b = nc.dram_tensor("x_b", [S, D], dt.float32, kind="ExternalInput").ap()
    wq8 = nc.dram_tensor("wq8", [D, DLOC], dt.float8e4, kind="ExternalInput").ap()
    wk8 = nc.dram_tensor("wk8", [D, DLOC], dt.float8e4, kind="ExternalInput").ap()
    wv8 = nc.dram_tensor("wv8", [D, DLOC], dt.float8e4, kind="ExternalInput").ap()
    wo8 = nc.dram_tensor("wo8", [D, D], dt.float8e4, kind="ExternalInput").ap()
    x_res = nc.dram_tensor(
        "x_res", [B, SLICE, D], dt.float32, kind="ExternalInput"
    ).ap()
    b_o = nc.dram_tensor("b_o", [D], dt.float32, kind="ExternalInput").ap()
    out_sl = nc.dram_tensor(
        "out_sl", [B, SLICE, D], dt.float32, kind="ExternalOutput"
    ).ap()
    if "g" in phases:
        xnt_dbg = nc.dram_tensor(
            "xnt_dbg", [128, 4, 8, 512], dt.float8e4, kind="ExternalOutput"
        ).ap()
        qk_dbg = nc.dram_tensor(
            "qk_dbg", [2, 2, 128, S], dt.bfloat16, kind="ExternalOutput"
        ).ap()
        vp_dbg = nc.dram_tensor(
            "vp_dbg", [128, HLOC, S // 128, 80], dt.float8e4, kind="ExternalOutput"
        ).ap()
        ain_dbg = nc.dram_tensor(
            "ain_dbg", [2, NCORES, 2 * DK, SLICE], dt.float8e4, kind="ExternalOutput"
        ).ap()
    if "h" in phases:
        av_dbg = nc.dram_tensor(
            "av_dbg", [2, DK + 1, 512], dt.float32, kind="ExternalOutput"
        ).ap()
        e2_dbg = nc.dram_tensor(
            "e2_dbg", [8, 128, 2, 2, 512], dt.float8e4, kind="ExternalOutput"
        ).ap()

    ST = S // 128  # 16 seq tiles
    FT = D // 128  # 8 feature tiles
    QC = S // 512  # 4 q-chunks for attention
    RT = B * SLICE // 128  # 4 row tiles of the output slice
    VP = 80  # padded V row stride (DoubleRow needs 16B-aligned steps)

    with tile.TileContext(nc) as tc:
        with (
            tc.tile_pool(name="singles", bufs=1) as singles,
            tc.tile_pool(name="persist", bufs=1) as persist,
            tc.tile_pool(name="dram", bufs=1, space="DRAM") as dram,
        ):
            eps_t = singles.tile([128, 1], dt.float32)
            nc.vector.memset(eps_t, EPS)
            ident = singles.tile([128, 128], dt.bfloat16)
            make_identity(nc, ident)

            # ---- persistent intermediates ----
            # x_norm^T in fp8: [128 part(d%128), chunk, f(d//128), 512 seq]
            xnt8_t = persist.tile([128, QC, FT, 512], dt.float8e4, tag="xnt8", name="xnt8")
            qT = [
                persist.tile([128, S], dt.bfloat16, tag=f"qT{m}", name=f"qT{m}")
                for m in range(2)
            ]
            kT = [
                persist.tile([128, S], dt.bfloat16, tag=f"kT{m}", name=f"kT{m}")
                for m in range(2)
            ]
            # V^T... actually V rows: [128 keys, h, kt, dk(+ones, pad 80)]
            vp8_t = persist.tile([128, HLOC, ST, VP], dt.float8e4, tag="vp8", name="vp8")
            nc.gpsimd.memset(vp8_t[:, :, :, DK : DK + 1], 1.0)

            # collective bounce buffers (fp8), one pair per head pair
            a2a_in = [
                dram.tile([NCORES, 2 * DK, SLICE], dt.float8e4, name=f"a2a_in{m}", tag=f"a2a_in{m}")
                for m in range(2)
            ]
            a2a_out = [
                dram.tile([NCORES, 2 * DK, SLICE], dt.float8e4, name=f"a2a_out{m}", tag=f"a2a_out{m}")
                for m in range(2)
            ]

            # weights
            wq_sb = singles.tile([128, FT, DLOC], dt.float8e4)
            wk_sb = singles.tile([128, FT, DLOC], dt.float8e4)
            wv_sb = singles.tile([128, FT, DLOC], dt.float8e4)
            b_bc = singles.tile([128, D], dt.float32)
            wo_sb = singles.tile([128, FT, D], dt.float8e4)
            xrb = singles.tile([128, RT, D], dt.float32)

            # ===== Phases A-C, software-pipelined =========================
            x_rows = x_b.rearrange("(t p) d -> t p d", p=128)
            with (
                tc.tile_pool(name="ln", bufs=8) as ln_pool,
                tc.tile_pool(name="lnst", bufs=12) as lnst,
                tc.tile_pool(name="epool", bufs=6) as epool,
                tc.tile_pool(name="aopool", bufs=4) as aopool,
                tc.tile_pool(name="ivpool", bufs=4) as ivpool,
            ):

                def emit_ln_chunk(c):
                    """Stage-batched LN for the chunk's 4 seq tiles: each
                    engine sees 4 independent items per stage, hiding the
                    cross-engine dependency latency."""
                    sts = list(range(4 * c, 4 * c + 4))
                    xts, mvs, rinvs, negmurs, xns, stgs = {}, {}, {}, {}, {}, {}
                    for st in sts:
                        x_t = ln_pool.tile([128, D], dt.float32, tag="x", name="x_t")
                        nc.sync.dma_start(out=x_t, in_=x_rows[st])
                        xts[st] = x_t
                    for st in sts:
                        stats = lnst.tile([128, 2, 6], dt.float32, tag="stats", name="stats")
                        for g in range(2):
                            nc.vector.bn_stats(
                                out=stats[:, g, :],
                                in_=xts[st][:, g * 512 : (g + 1) * 512],
                            )
                        mv = lnst.tile([128, 2], dt.float32, tag="mv", name="mv")
                        nc.vector.bn_aggr(out=mv, in_=stats)
                        mvs[st] = mv
                    sds = {}
                    for st in sts:
                        sd = lnst.tile([128, 1], dt.float32, tag="sd", name="sd")
                        nc.scalar.activation(
                            out=sd, in_=mvs[st][:, 1:2], func=AF.Sqrt,
                            bias=eps_t, scale=1.0,
                        )
                        sds[st] = sd
                    for st in sts:
                        rinv = lnst.tile([128, 1], dt.float32, tag="rinv", name="rinv")
                        nc.vector.reciprocal_approx_fast(out=rinv, in_=sds[st])
                        rinvs[st] = rinv
                    for st in sts:
                        negmur = lnst.tile([128, 1], dt.float32, tag="negmur", name="negmur")
                        nc.vector.tensor_scalar(
                            out=negmur,
                            in0=mvs[st][:, 0:1],
                            scalar1=rinvs[st],
                            scalar2=-1.0,
                            op0=OP.mult,
                            op1=OP.mult,
                        )
                        negmurs[st] = negmur
                    for st in sts:
                        xn = ln_pool.tile([128, D], dt.bfloat16, tag="xn", name="xn")
                        nc.scalar.activation(
                            out=xn, in_=xts[st], func=AF.Identity,
                            bias=negmurs[st], scale=rinvs[st],
                        )
                        xns[st] = xn
                    # PE transposes (per 128x128 f-tile) with the fp8 cast
                    # folded into the PSUM eviction
                    for st in sts:
                        st4 = st % 4
                        for fp in range(FT // 2):
                            tr_ps = ps_tr.tile([128, 256], dt.bfloat16, tag="tr", name="tr")
                            for k in range(2):
                                nc.tensor.transpose(
                                    tr_ps[:, k * 128 : (k + 1) * 128],
                                    xns[st][:, (2 * fp + k) * 128 : (2 * fp + k + 1) * 128],
                                    ident,
                                )
                            dst = xnt8_t[:, c, 2 * fp : 2 * fp + 2, st4 * 128 : (st4 + 1) * 128]
                            src = tr_ps.rearrange("p (k q) -> p k q", k=2)
                            if fp % 2 == 0:
                                nc.scalar.copy(out=dst, in_=src)
                            else:
                                nc.vector.tensor_copy(out=dst, in_=src)

                def emit_kq(w_sb, dst, mt, ch, evict_eng):
                    ps = ps_proj.tile([128, 512], dt.float32, tag="qkv", name="kq_ps")
                    for fp in range(FT // 2):
                        nc.tensor.matmul(
                            ps,
                            lhsT=w_sb[:, 2 * fp : 2 * fp + 2, mt * 128 : (mt + 1) * 128],
                            rhs=xnt8_t[:, ch, 2 * fp : 2 * fp + 2, :],
                            start=(fp == 0),
                            stop=(fp == FT // 2 - 1),
                            perf_mode=DR,
                        )
                    if evict_eng == "act":
                        nc.scalar.copy(
                            out=dst[mt][:, ch * 512 : (ch + 1) * 512], in_=ps
                        )
                    else:
                        nc.vector.tensor_copy(
                            out=dst[mt][:, ch * 512 : (ch + 1) * 512], in_=ps
                        )

                def emit_v(st):
                    c, st4 = st // 4, st % 4
                    ps = ps_proj.tile([128, DLOC], dt.float32, tag="qkv", name="v_ps")
                    for fp in range(FT // 2):
                        nc.tensor.matmul(
                            ps,
                            lhsT=xnt8_t[
                                :, c, 2 * fp : 2 * fp + 2, st4 * 128 : (st4 + 1) * 128
                            ],
                            rhs=wv_sb[:, 2 * fp : 2 * fp + 2, :],
                            start=(fp == 0),
                            stop=(fp == FT // 2 - 1),
                            perf_mode=DR,
                        )
                    nc.scalar.copy(
                        out=vp8_t[:, :, st, 0:DK],
                        in_=ps.rearrange("p (h d) -> p h d", h=HLOC),
                    )

                # -- block loop: LN -> K/Q (chunk c) -> V ---------------
                ps_proj_cm = tc.tile_pool(name="ps_proj", bufs=2, space="PSUM")
                ps_proj = ps_proj_cm.__enter__()
                ps_tr_cm = tc.tile_pool(name="ps_tr", bufs=2, space="PSUM")
                ps_tr = ps_tr_cm.__enter__()
                emit_ln_chunk(0)
                nc.sync.dma_start(
                    out=wk_sb, in_=wk8.rearrange("(t p) m -> p t m", p=128)
                )
                nc.sync.dma_start(
                    out=wq_sb, in_=wq8.rearrange("(t p) m -> p t m", p=128)
                )
                nc.sync.dma_start(
                    out=wv_sb, in_=wv8.rearrange("(t p) m -> p t m", p=128)
                )
                for c in range(4):
                    # 1-deep pipeline: next chunk's LN chain advances while
                    # this chunk's projections run on the PE
                    if c + 1 < 4:
                        emit_ln_chunk(c + 1)
                    for mt in range(2):
                        emit_kq(wk_sb, kT, mt, c, "act")
                    for mt in range(2):
                        emit_kq(wq_sb, qT, mt, c, "dve")
                    for st in range(4 * c, 4 * c + 4):
                        emit_v(st)
                    if c == 1:
                        nc.sync.dma_start(
                            out=b_bc,
                            in_=bass.AP(
                                tensor=b_o.tensor,
                                offset=b_o.offset,
                                ap=[[0, 128]] + list(b_o.ap),
                            ),
                        )
                        nc.sync.dma_start(
                            out=wo_sb, in_=wo8.rearrange("(t p) m -> p t m", p=128)
                        )
                        nc.sync.dma_start(
                            out=xrb,
                            in_=x_res.rearrange("b (t p) d -> p (b t) d", p=128),
                        )
                        for t in range(RT):
                            nc.vector.tensor_add(
                                out=xrb[:, t, :], in0=xrb[:, t, :], in1=b_bc
                            )
                ps_tr_cm.__exit__(None, None, None)
                ps_proj_cm.__exit__(None, None, None)

                # -- attention, head-pair-major + split AllToAll --------
                ps_s_cm = tc.tile_pool(name="ps_s", bufs=3, space="PSUM")
                ps_s = ps_s_cm.__enter__()
                ps_av_cm = tc.tile_pool(name="ps_av", bufs=1, space="PSUM")
                ps_av = ps_av_cm.__enter__()
                for hp in range(2):
                    for qc in range(QC):
                        av = [
                            ps_av.tile(
                                [DK + 1, 512],
                                dt.float32,
                                tag=f"av{j}",
                                name=f"av{hp}{j}",
                            )
                            for j in range(2)
                        ]
                        # AV trails scores/exp by one kt-pair so the PE never
                        # head-of-line blocks on an unfinished exp
                        e2s = {}
                        for m in range(ST // 2 + 2):
                            if m < ST // 2:
                                e2 = epool.tile(
                                    [128, 2, 2, 512], dt.float8e4, tag="e2", name="e2"
                                )
                                e2s[m] = e2
                                for par in range(2):
                                    kt = 2 * m + par
                                    s_ps = ps_s.tile(
                                        [128, 1024], dt.float32, tag="s", name="s_ps"
                                    )
                                    for j in range(2):
                                        nc.tensor.matmul(
                                            s_ps[:, j * 512 : (j + 1) * 512],
                                            lhsT=kT[hp][
                                                j * 64 : (j + 1) * 64,
                                                kt * 128 : (kt + 1) * 128,
                                            ],
                                            rhs=qT[hp][
                                                j * 64 : (j + 1) * 64,
                                                qc * 512 : (qc + 1) * 512,
                                            ],
                                            start=True,
                                            stop=True,
                                        )
                                    if kt % 8 < 5:
                                        # exact exp on ScalarE, fp8 out
                                        nc.scalar.activation(
                                            out=e2[:, par, :, :],
                                            in_=s_ps,
                                            func=AF.Exp,
                                            scale=float(C_EXP),
                                        )
                                    else:
                                        # exp2 bit-trick on VectorE: fp8e4 bits
                                        # = round(s*A8 + B8), saturating at 0
                                        nc.vector.tensor_scalar(
                                            out=e2[:, par, :, :].bitcast(dt.uint8),
                                            in0=s_ps,
                                            scalar1=float(A8),
                                            scalar2=float(B8),
                                            op0=OP.mult,
                                            op1=OP.add,
                                        )
                                if "h" in phases and hp == 0 and qc == 0:
                                    nc.sync.dma_start(out=e2_dbg[m], in_=e2)
                            if m >= 2:
                                mm = m - 2
                                for j in range(2):
                                    nc.tensor.matmul(
                                        av[j],
                                        lhsT=vp8_t[
                                            :, 2 * hp + j, 2 * mm : 2 * mm + 2, 0 : DK + 1
                                        ],
                                        rhs=e2s[mm][:, :, j, :],
                                        start=(mm == 0),
                                        stop=(mm == ST // 2 - 1),
                                        perf_mode=DR,
                                    )
                        if "h" in phases and hp == 0 and qc == 0:
                            for j in range(2):
                                avc = aopool.tile(
                                    [DK + 1, 512], dt.float32, tag="avc", name="avc"
                                )
                                nc.vector.tensor_copy(out=avc, in_=av[j])
                                nc.sync.dma_start(out=av_dbg[j], in_=avc)
                        # normalize + evict (DVE recip+mult from PSUM,
                        # gpsimd broadcast; gpsimd/DMA can't read PSUM)
                        for j in range(2):
                            # den must be copied to a partition-0 SBUF tile:
                            # recip straight off the partition-64 PSUM row
                            # silently reads partition 0
                            den = ivpool.tile([1, 512], dt.float32, tag="den", name="den")
                            nc.vector.tensor_copy(out=den, in_=av[j][DK : DK + 1, :])
                            invd = ivpool.tile([1, 512], dt.float32, tag="invd", name="invd")
                            nc.vector.reciprocal_approx_fast(out=invd, in_=den)
                            ibc = ivpool.tile([DK, 512], dt.float32, tag="ibc", name="ibc")
                            nc.gpsimd.partition_broadcast(ibc, invd)
                            ao = aopool.tile([DK, 512], dt.float8e4, tag="ao", name="ao")
                            nc.vector.tensor_tensor(
                                out=ao, in0=av[j][0:DK, :], in1=ibc, op=OP.mult
                            )
                            for half in range(2):
                                nc.sync.dma_start(
                                    out=a2a_in[hp][
                                        2 * qc + half, j * DK : (j + 1) * DK, :
                                    ],
                                    in_=ao[:, half * 256 : (half + 1) * 256],
                                )
                    if "g" in phases:
                        nc.sync.dma_start(out=ain_dbg[hp], in_=a2a_in[hp])
                    if "D" in phases:
                        nc.gpsimd.collective_compute(
                            "AllToAll",
                            mybir.AluOpType.bypass,
                            replica_groups=[list(range(NCORES))],
                            ins=[a2a_in[hp].opt()],
                            outs=[a2a_out[hp].opt()],
                        )

                ps_av_cm.__exit__(None, None, None)
                ps_s_cm.__exit__(None, None, None)
                if "g" in phases:
                    nc.sync.dma_start(out=xnt_dbg, in_=xnt8_t)
                    for m in range(2):
                        nc.sync.dma_start(out=qk_dbg[0, m], in_=qT[m])
                        nc.sync.dma_start(out=qk_dbg[1, m], in_=kT[m])
                    nc.sync.dma_start(out=vp_dbg, in_=vp8_t)

            # ============ Phase E: output projection ======================
            # gathered slot r of a2a_out[hp] = heads {4i+2hp, 4i+2hp+1} of
            # group i = r%4, batch r//4 -> orig f-tile 2*(r%4)+hp; wo_sb is
            # host-permuted hp-major: slot v = 4*hp + i4
            if "E" in phases:
                with (
                    tc.tile_pool(name="ps_wo", bufs=1, space="PSUM") as ps_wo,
                    tc.tile_pool(name="attg", bufs=1) as attg_pool,
                    tc.tile_pool(name="outp", bufs=4) as outp,
                ):
                    wo_ps = {}
                    for b in range(B):
                        for mt in range(SLICE // 128):
                            for oc in range(2):
                                wo_ps[b, mt, oc] = ps_wo.tile(
                                    [128, 512],
                                    dt.float32,
                                    tag=f"wo{b}{mt}{oc}",
                                    name=f"wo{b}{mt}{oc}",
                                )
                    attg = {}
                    for hp in range(2):
                        for b in range(B):
                            ag = attg_pool.tile(
                                [128, 4, SLICE],
                                dt.float8e4,
                                tag=f"ag{hp}{b}",
                                name=f"ag{hp}{b}",
                            )
                            attg[hp, b] = ag
                            nc.sync.dma_start(
                                out=ag,
                                in_=a2a_out[hp][4 * b : 4 * (b + 1), :, :].rearrange(
                                    "s (t p) q -> p (s t) q", p=128
                                ),
                            )
                    for hp in range(2):
                        for b in range(B):
                            for mt in range(SLICE // 128):
                                for oc in range(2):
                                    for u in range(2):
                                        nc.tensor.matmul(
                                            wo_ps[b, mt, oc],
                                            lhsT=attg[hp, b][
                                                :, 2 * u : 2 * u + 2,
                                                mt * 128 : (mt + 1) * 128,
                                            ],
                                            rhs=wo_sb[
                                                :,
                                                4 * hp + 2 * u : 4 * hp + 2 * u + 2,
                                                oc * 512 : (oc + 1) * 512,
                                            ],
                                            start=(hp == 0 and u == 0),
                                            stop=(hp == 1 and u == 1),
                                            perf_mode=DR,
                                        )
                    for b in range(B):
                        for mt in range(SLICE // 128):
                            for oc in range(2):
                                o_t = outp.tile([128, 512], dt.float32, tag="o")
                                # out = psum/1024 + (residual + bias)
                                nc.vector.scalar_tensor_tensor(
                                    out=o_t,
                                    in0=wo_ps[b, mt, oc],
                                    scalar=float(1.0 / (WS * WS)),
                                    in1=xrb[
                                        :,
                                        b * (SLICE // 128) + mt,
                                        oc * 512 : (oc + 1) * 512,
                                    ],
                                    op0=OP.mult,
                                    op1=OP.add,
                                )
                                nc.sync.dma_start(
                                    out=out_sl[
                                        b,
                                        mt * 128 : (mt + 1) * 128,
                                        oc * 512 : (oc + 1) * 512,
                                    ],
                                    in_=o_t,
                                )
            else:
                nc.sync.dma_start(out=out_sl[:, :, :], in_=x_res[:, :, :])

    nc.compile()
    return nc


def _get_nc(phases="ABCDE"):
    key = ("nc", phases)
    if key not in _CACHE:
        _CACHE[key] = _build(phases)
    return _CACHE[key]


def _make_in_maps(inputs):
    x = np.asarray(inputs["x"], np.float32)
    w_q = np.asarray(inputs["w_q"], np.float32)
    w_k = np.asarray(inputs["w_k"], np.float32)
    w_v = np.asarray(inputs["w_v"], np.float32)
    w_o = np.asarray(inputs["w_o"], np.float32)
    b_o = np.asarray(inputs["b_o"], np.float32)
    gamma = np.asarray(inputs["ln_gamma"], np.float32)
    beta = np.asarray(inputs["ln_beta"], np.float32)

    assert np.allclose(beta, 0.0), "nonzero ln_beta not supported"
    # hp-major f-tile permutation for DoubleRow pairing in phase E
    woT = np.ascontiguousarray(w_o.T) * WS
    woT_p = np.concatenate(
        [woT[128 * (2 * (v % 4) + v // 4) : 128 * (2 * (v % 4) + v // 4) + 128] for v in range(8)],
        axis=0,
    ).astype(F8)
    # LN gamma folds exactly into the input side of the QKV projections
    w_qg = w_q * gamma[None, :] * WS
    w_kg = w_k * gamma[None, :] * WS
    w_vg = w_v * gamma[None, :] * WS
    in_maps = []
    for r in range(NCORES):
        b, i = r // 4, r % 4
        sl = slice(DLOC * i, DLOC * (i + 1))
        in_maps.append(
            {
                "x_b": np.ascontiguousarray(x[b]),
                "wq8": np.ascontiguousarray(w_qg[sl].T).astype(F8),
                "wk8": np.ascontiguousarray(w_kg[sl].T).astype(F8),
                "wv8": np.ascontiguousarray(w_vg[sl].T).astype(F8),
                "wo8": woT_p,
                "x_res": np.ascontiguousarray(x[:, SLICE * r : SLICE * (r + 1), :]),
                "b_o": b_o,
            }
        )
    return in_maps


def _install_ntff_hook():
    """The agent image's antenv lacks axon_hooks; recreate it so
    trace=True can capture NTFF profiles through libaxon_pjrt.so."""
    import types

    from concourse import bass_utils

    if "antenv.axon_hooks" not in sys.modules:
        import antenv
        from trn_agent_boot.trn_boot import _ntff_profile_via_ctypes

        mod = types.ModuleType("antenv.axon_hooks")
        state = {}
        mod.set_axon_ntff_profile_hook = lambda h: state.update(h=h)
        mod.get_axon_ntff_profile_hook = lambda: state.get("h")
        sys.modules["antenv.axon_hooks"] = mod
        antenv.axon_hooks = mod
        mod.set_axon_ntff_profile_hook(
            _ntff_profile_via_ctypes("/opt/axon/libaxon_pjrt.so")
        )
    bass_utils.upload_artifacts = lambda tmpdir: tmpdir


def run(inputs, trace=False, phases="ABCDE", tmpdir=None, trace_cores=None):
    from concourse import bass_utils

    if trace:
        _install_ntff_hook()
    nc = _get_nc(phases)
    in_maps = _make_in_maps(inputs)
    res = bass_utils.run_bass_kernel_spmd(
        nc,
        in_maps,
        core_ids=list(range(NCORES)),
        trace=trace,
        tmpdir=tmpdir,
        trace_cores=trace_cores,
    )
    out = np.empty((B, S, D), np.float32)
    for r in range(NCORES):
        out[:, SLICE * r : SLICE * (r + 1), :] = res.results[r]["out_sl"]
    return out, res


def kernel(**inputs):
    out, _ = run(inputs)
    return out


# revision 51
# speedup vs baseline: 1.3174x; 1.0331x over previous
"""Multi-head attention (pre-LN + residual) on 8 trn2 NeuronCores.

Sharding: core r = (batch b = r//4, head group i = r%4, 4 heads each).

Per core: stage-batched LN over 4-seq-tile chunks (1-deep chunk
pipeline) -> PE transpose of x_norm with the fp8e4 cast folded into
the PSUM eviction -> fp8 DoubleRow K/Q/V projections (weights
pre-scaled x32 on host to dodge e4m3 subnormals; Q/K evicted x32 in
bf16, V x32 in fp8 with a padded-80 row stride and a ones column) ->
scores^T = K Q^T per head (bf16, 64-contraction) -> exp split 10/6
between ScalarE (exact exp, fp8e4 out) and VectorE (exp2 bit-trick:
fp8e4 bit pattern = round(psum*A8+B8) via uint8 store w/ saturation;
softmax ratio cancels the shared approximation error) -> AV in fp8
DoubleRow over kt-pairs, software-pipelined to trail scores/exp by 2
kt-pairs (ones column gives the denominator) -> normalize (ACT den
copy, DVE recip+mult, gpsimd broadcast) -> fp8 AllToAll per head
pair, the first overlapped under the second pair's attention -> fp8
DoubleRow w_o matmul against hp-major-permuted w_o (+1/1024 rescale
fused with bias+residual via scalar_tensor_tensor) producing a
256-row seq slice of both batches.
"""

import sys

sys.path.insert(0, "/opt/trn_rl_repo")

import numpy as np
import ml_dtypes

BF16 = ml_dtypes.bfloat16
F8 = ml_dtypes.float8_e4m3fn

# Problem constants (hardcoded per contract)
B = 2
S = 2048
D = 1024
H = 16
DK = 64
NCORES = 8
HLOC = 4  # heads per core
DLOC = HLOC * DK  # 256
SLICE = S // NCORES  # 256 output rows per batch per core
EPS = 1e-5
WS = 32.0  # fp8 weight scale (avoids e4m3 subnormals)
# scores psum = (32Q)(32K) = 1024*QK ; softmax wants exp(QK/8)
C_EXP = 1.0 / (8.0 * WS * WS)
LOG2E = 1.4426950408889634
CORR = -0.045
A8 = C_EXP * LOG2E * 8.0
B8 = (7.0 + CORR) * 8.0

_CACHE = {}


def _build(phases="ABCDE"):
    import concourse.bass as bass
    import concourse.mybir as mybir
    import concourse.tile as tile
    from concourse import bacc

    from concourse.masks import make_identity

    dt = mybir.dt
    AF = mybir.ActivationFunctionType
    OP = mybir.AluOpType
    DR = mybir.MatmulPerfMode.DoubleRow

    nc = bacc.Bacc(
        "TRN2",
        target_bir_lowering=False,
        debug=False,
        enable_asserts=False,
        num_devices=NCORES,
    )

    # ---- I/O ----
    x_b = nc.dram_tensor("x_b", [S, D], dt.float32, kind="ExternalInput").ap()
    wq8 = nc.dram_tensor("wq8", [D, DLOC], dt.float8e4, kind="ExternalInput").ap()
    wk8 = nc.dram_tensor("wk8", [D, DLOC], dt.float8e4, kind="ExternalInput").ap()
    wv8 = nc.dram_tensor("wv8", [D, DLOC], dt.float8e4, kind="ExternalInput").ap()
    wo8 = nc.dram_tensor("wo8", [D, D], dt.float8e4, kind="ExternalInput").ap()
    x_res = nc.dram_tensor(
        "x_res", [B, SLICE, D], dt.float32, kind="ExternalInput"
    ).ap()
    out_sl = nc.dram_tensor(
        "out_sl", [B, SLICE, D], dt.float32, kind="ExternalOutput"
    ).ap()
    if "g" in phases:
        xnt_dbg = nc.dram_tensor(
            "xnt_dbg", [128, 4, 8, 512], dt.float8e4, kind="ExternalOutput"
        ).ap()
        qk_dbg = nc.dram_tensor(
            "qk_dbg", [2, 2, 128, S], dt.bfloat16, kind="ExternalOutput"
        ).ap()
        vp_dbg = nc.dram_tensor(
            "vp_dbg", [128, HLOC, S // 128, 80], dt.float8e4, kind="ExternalOutput"
        ).ap()
        ain_dbg = nc.dram_tensor(
            "ain_dbg", [2, NCORES, 2 * DK, SLICE], dt.float8e4, kind="ExternalOutput"
        ).ap()
    if "h" in phases:
        av_dbg = nc.dram_tensor(
            "av_dbg", [2, DK + 1, 512], dt.float32, kind="ExternalOutput"
        ).ap()
        e2_dbg = nc.dram_tensor(
            "e2_dbg", [8, 128, 2, 2, 512], dt.float8e4, kind="ExternalOutput"
        ).ap()

    ST = S // 128  # 16 seq tiles
    FT = D // 128  # 8 feature tiles
    QC = S // 512  # 4 q-chunks for attention
    RT = B * SLICE // 128  # 4 row tiles of the output slice
    VP = 80  # padded V row stride (DoubleRow needs 16B-aligned steps)

    with tile.TileContext(nc) as tc:
        with (
            tc.tile_pool(name="singles", bufs=1) as singles,
            tc.tile_pool(name="persist", bufs=1) as persist,
            tc.tile_pool(name="dram", bufs=1, space="DRAM") as dram,
        ):
            eps_t = singles.tile([128, 1], dt.float32)
            nc.vector.memset(eps_t, EPS)
            ident = singles.tile([128, 128], dt.bfloat16)
            make_identity(nc, ident)

            # ---- persistent intermediates ----
            # x_norm^T in fp8: [128 part(d%128), chunk, f(d//128), 512 seq]
            xnt8_t = persist.tile([128, QC, FT, 512], dt.float8e4, tag="xnt8", name="xnt8")
            qT = [
                persist.tile([128, S], dt.bfloat16, tag=f"qT{m}", name=f"qT{m}")
                for m in range(2)
            ]
            kT = [
                persist.tile([128, S], dt.bfloat16, tag=f"kT{m}", name=f"kT{m}")
                for m in range(2)
            ]
            # V^T... actually V rows: [128 keys, h, kt, dk(+ones, pad 80)]
            vp8_t = persist.tile([128, HLOC, ST, VP], dt.float8e4, tag="vp8", name="vp8")
            nc.gpsimd.memset(vp8_t[:, :, :, DK : DK + 1], 1.0)

            # collective bounce buffers (fp8), one pair per head pair
            a2a_in = [
                dram.tile([NCORES, 2 * DK, SLICE], dt.float8e4, name=f"a2a_in{m}", tag=f"a2a_in{m}")
                for m in range(2)
            ]
            a2a_out = [
                dram.tile([NCORES, 2 * DK, SLICE], dt.float8e4, name=f"a2a_out{m}", tag=f"a2a_out{m}")
                for m in range(2)
            ]

            # weights
            wq_sb = singles.tile([128, FT, DLOC], dt.float8e4)
            wk_sb = singles.tile([128, FT, DLOC], dt.float8e4)
            wv_sb = singles.tile([128, FT, DLOC], dt.float8e4)
            wo_sb = singles.tile([128, FT, D], dt.float8e4)
            xrb = singles.tile([128, RT, D], dt.float32)

            # ===== Phases A-C, software-pipelined =========================
            x_rows = x_b.rearrange("(t p) d -> t p d", p=128)
            with (
                tc.tile_pool(name="ln", bufs=8) as ln_pool,
                tc.tile_pool(name="lnst", bufs=12) as lnst,
                tc.tile_pool(name="epool", bufs=6) as epool,
                tc.tile_pool(name="aopool", bufs=4) as aopool,
                tc.tile_pool(name="ivpool", bufs=4) as ivpool,
            ):

                def emit_ln_chunk(c):
                    """Stage-batched LN for the chunk's 4 seq tiles: each
                    engine sees 4 independent items per stage, hiding the
                    cross-engine dependency latency."""
                    sts = list(range(4 * c, 4 * c + 4))
                    xts, mvs, rinvs, negmurs, xns, stgs = {}, {}, {}, {}, {}, {}
                    for st in sts:
                        x_t = ln_pool.tile([128, D], dt.float32, tag="x", name="x_t")
                        nc.sync.dma_start(out=x_t, in_=x_rows[st])
                        xts[st] = x_t
                    for st in sts:
                        stats = lnst.tile([128, 2, 6], dt.float32, tag="stats", name="stats")
                        for g in range(2):
                            nc.vector.bn_stats(
                                out=stats[:, g, :],
                                in_=xts[st][:, g * 512 : (g + 1) * 512],
                            )
                        mv = lnst.tile([128, 2], dt.float32, tag="mv", name="mv")
                        nc.vector.bn_aggr(out=mv, in_=stats)
                        mvs[st] = mv
                    sds = {}
                    for st in sts:
                        sd = lnst.tile([128, 1], dt.float32, tag="sd", name="sd")
                        nc.scalar.activation(
                            out=sd, in_=mvs[st][:, 1:2], func=AF.Sqrt,
                            bias=eps_t, scale=1.0,
                        )
                        sds[st] = sd
                    for st in sts:
                        rinv = lnst.tile([128, 1], dt.float32, tag="rinv", name="rinv")
                        nc.vector.reciprocal_approx_fast(out=rinv, in_=sds[st])
                        rinvs[st] = rinv
                    for st in sts:
                        negmur = lnst.tile([128, 1], dt.float32, tag="negmur", name="negmur")
                        nc.vector.tensor_scalar(
                            out=negmur,
                            in0=mvs[st][:, 0:1],
                            scalar1=rinvs[st],
                            scalar2=-1.0,
                            op0=OP.mult,
                            op1=OP.mult,
                        )
                        negmurs[st] = negmur
                    for st in sts:
                        xn = ln_pool.tile([128, D], dt.bfloat16, tag="xn", name="xn")
                        nc.scalar.activation(
                            out=xn, in_=xts[st], func=AF.Identity,
                            bias=negmurs[st], scale=rinvs[st],
                        )
                        xns[st] = xn
                    # PE transposes (per 128x128 f-tile) with the fp8 cast
                    # folded into the PSUM eviction
                    for st in sts:
                        st4 = st % 4
                        for fp in range(FT // 2):
                            tr_ps = ps_tr.tile([128, 256], dt.bfloat16, tag="tr", name="tr")
                            for k in range(2):
                                nc.tensor.transpose(
                                    tr_ps[:, k * 128 : (k + 1) * 128],
                                    xns[st][:, (2 * fp + k) * 128 : (2 * fp + k + 1) * 128],
                                    ident,
                                )
                            dst = xnt8_t[:, c, 2 * fp : 2 * fp + 2, st4 * 128 : (st4 + 1) * 128]
                            src = tr_ps.rearrange("p (k q) -> p k q", k=2)
                            if fp % 2 == 0:
                                nc.scalar.copy(out=dst, in_=src)
                            else:
                                nc.vector.tensor_copy(out=dst, in_=src)

                def emit_kq(w_sb, dst, mt, ch, evict_eng):
                    ps = ps_proj.tile([128, 512], dt.float32, tag="qkv", name="kq_ps")
                    for fp in range(FT // 2):
                        nc.tensor.matmul(
                            ps,
                            lhsT=w_sb[:, 2 * fp : 2 * fp + 2, mt * 128 : (mt + 1) * 128],
                            rhs=xnt8_t[:, ch, 2 * fp : 2 * fp + 2, :],
                            start=(fp == 0),
                            stop=(fp == FT // 2 - 1),
                            perf_mode=DR,
                        )
                    if evict_eng == "act":
                        nc.scalar.copy(
                            out=dst[mt][:, ch * 512 : (ch + 1) * 512], in_=ps
                        )
                    else:
                        nc.vector.tensor_copy(
                            out=dst[mt][:, ch * 512 : (ch + 1) * 512], in_=ps
                        )

                def emit_v(st):
                    c, st4 = st // 4, st % 4
                    ps = ps_proj.tile([128, DLOC], dt.float32, tag="qkv", name="v_ps")
                    for fp in range(FT // 2):
                        nc.tensor.matmul(
                            ps,
                            lhsT=xnt8_t[
                                :, c, 2 * fp : 2 * fp + 2, st4 * 128 : (st4 + 1) * 128
                            ],
                            rhs=wv_sb[:, 2 * fp : 2 * fp + 2, :],
                            start=(fp == 0),
                            stop=(fp == FT // 2 - 1),
                            perf_mode=DR,
                        )
                    nc.scalar.copy(
                        out=vp8_t[:, :, st, 0:DK],
                        in_=ps.rearrange("p (h d) -> p h d", h=HLOC),
                    )

                # -- block loop: LN -> K/Q (chunk c) -> V ---------------
                ps_proj_cm = tc.tile_pool(name="ps_proj", bufs=2, space="PSUM")
                ps_proj = ps_proj_cm.__enter__()
                ps_tr_cm = tc.tile_pool(name="ps_tr", bufs=2, space="PSUM")
                ps_tr = ps_tr_cm.__enter__()
                emit_ln_chunk(0)
                nc.sync.dma_start(
                    out=wk_sb, in_=wk8.rearrange("(t p) m -> p t m", p=128)
                )
                nc.sync.dma_start(
                    out=wq_sb, in_=wq8.rearrange("(t p) m -> p t m", p=128)
                )
                nc.sync.dma_start(
                    out=wv_sb, in_=wv8.rearrange("(t p) m -> p t m", p=128)
                )
                for c in range(4):
                    # 1-deep pipeline: next chunk's LN chain advances while
                    # this chunk's projections run on the PE
                    if c + 1 < 4:
                        emit_ln_chunk(c + 1)
                    for mt in range(2):
                        emit_kq(wk_sb, kT, mt, c, "act")
                    for mt in range(2):
                        emit_kq(wq_sb, qT, mt, c, "dve")
                    for st in range(4 * c, 4 * c + 4):
                        emit_v(st)
                    if c == 1:
                        # x_res arrives with b_o pre-added on the host
                        nc.sync.dma_start(
                            out=wo_sb, in_=wo8.rearrange("(t p) m -> p t m", p=128)
                        )
                        nc.sync.dma_start(
                            out=xrb,
                            in_=x_res.rearrange("b (t p) d -> p (b t) d", p=128),
                        )
                ps_tr_cm.__exit__(None, None, None)
                ps_proj_cm.__exit__(None, None, None)

                # -- attention, head-pair-major + split AllToAll --------
                ps_s_cm = tc.tile_pool(name="ps_s", bufs=3, space="PSUM")
                ps_s = ps_s_cm.__enter__()
                ps_av_cm = tc.tile_pool(name="ps_av", bufs=1, space="PSUM")
                ps_av = ps_av_cm.__enter__()
                for hp in range(2):
                    for qc in range(QC):
                        av = [
                            ps_av.tile(
                                [DK + 1, 512],
                                dt.float32,
                                tag=f"av{j}",
                                name=f"av{hp}{j}",
                            )
                            for j in range(2)
                        ]
                        # AV trails scores/exp by one kt-pair so the PE never
                        # head-of-line blocks on an unfinished exp
                        e2s = {}
                        for m in range(ST // 2 + 2):
                            if m < ST // 2:
                                e2 = epool.tile(
                                    [128, 2, 2, 512], dt.float8e4, tag="e2", name="e2"
                                )
                                e2s[m] = e2
                                for par in range(2):
                                    kt = 2 * m + par
                                    s_ps = ps_s.tile(
                                        [128, 1024], dt.float32, tag="s", name="s_ps"
                                    )
                                    for j in range(2):
                                        nc.tensor.matmul(
                                            s_ps[:, j * 512 : (j + 1) * 512],
                                            lhsT=kT[hp][
                                                j * 64 : (j + 1) * 64,
                                                kt * 128 : (kt + 1) * 128,
                                            ],
                                            rhs=qT[hp][
                                                j * 64 : (j + 1) * 64,
                                                qc * 512 : (qc + 1) * 512,
                                            ],
                                            start=True,
                                            stop=True,
                                        )
                                    if kt % 8 < 5:
                                        # exact exp on ScalarE, fp8 out
                                        nc.scalar.activation(
                                            out=e2[:, par, :, :],
                                            in_=s_ps,
                                            func=AF.Exp,
                                            scale=float(C_EXP),
                                        )
                                    else:
                                        # exp2 bit-trick on VectorE: fp8e4 bits
                                        # = round(s*A8 + B8), saturating at 0
                                        nc.vector.tensor_scalar(
                                            out=e2[:, par, :, :].bitcast(dt.uint8),
                                            in0=s_ps,
                                            scalar1=float(A8),
                                            scalar2=float(B8),
                                            op0=OP.mult,
                                            op1=OP.add,
                                        )
                                if "h" in phases and hp == 0 and qc == 0:
                                    nc.sync.dma_start(out=e2_dbg[m], in_=e2)
                            if m >= 2:
                                mm = m - 2
                                for j in range(2):
                                    nc.tensor.matmul(
                                        av[j],
                                        lhsT=vp8_t[
                                            :, 2 * hp + j, 2 * mm : 2 * mm + 2, 0 : DK + 1
                                        ],
                                        rhs=e2s[mm][:, :, j, :],
                                        start=(mm == 0),
                                        stop=(mm == ST // 2 - 1),
                                        perf_mode=DR,
                                    )
                        if "h" in phases and hp == 0 and qc == 0:
                            for j in range(2):
                                avc = aopool.tile(
                                    [DK + 1, 512], dt.float32, tag="avc", name="avc"
                                )
                                nc.vector.tensor_copy(out=avc, in_=av[j])
                                nc.sync.dma_start(out=av_dbg[j], in_=avc)
                        # normalize + evict (DVE recip+mult from PSUM,
                        # gpsimd broadcast; gpsimd/DMA can't read PSUM)
                        for j in range(2):
                            # den must be copied to a partition-0 SBUF tile:
                            # recip straight off the partition-64 PSUM row
                            # silently reads partition 0
                            den = ivpool.tile([1, 512], dt.float32, tag="den", name="den")
                            nc.vector.tensor_copy(out=den, in_=av[j][DK : DK + 1, :])
                            invd = ivpool.tile([1, 512], dt.float32, tag="invd", name="invd")
                            nc.vector.reciprocal_approx_fast(out=invd, in_=den)
                            ibc = ivpool.tile([DK, 512], dt.float32, tag="ibc", name="ibc")
                            nc.gpsimd.partition_broadcast(ibc, invd)
                            ao = aopool.tile([DK, 512], dt.float8e4, tag="ao", name="ao")
                            nc.vector.tensor_tensor(
                                out=ao, in0=av[j][0:DK, :], in1=ibc, op=OP.mult
                            )
                            for half in range(2):
                                nc.sync.dma_start(
                                    out=a2a_in[hp][
                                        2 * qc + half, j * DK : (j + 1) * DK, :
                                    ],
                                    in_=ao[:, half * 256 : (half + 1) * 256],
                                )
                    if "g" in phases:
                        nc.sync.dma_start(out=ain_dbg[hp], in_=a2a_in[hp])
                    if "D" in phases:
                        nc.gpsimd.collective_compute(
                            "AllToAll",
                            mybir.AluOpType.bypass,
                            replica_groups=[list(range(NCORES))],
                            ins=[a2a_in[hp].opt()],
                            outs=[a2a_out[hp].opt()],
                        )

                ps_av_cm.__exit__(None, None, None)
                ps_s_cm.__exit__(None, None, None)
                if "g" in phases:
                    nc.sync.dma_start(out=xnt_dbg, in_=xnt8_t)
                    for m in range(2):
                        nc.sync.dma_start(out=qk_dbg[0, m], in_=qT[m])
                        nc.sync.dma_start(out=qk_dbg[1, m], in_=kT[m])
                    nc.sync.dma_start(out=vp_dbg, in_=vp8_t)

            # ============ Phase E: output projection ======================
            # gathered slot r of a2a_out[hp] = heads {4i+2hp, 4i+2hp+1} of
            # group i = r%4, batch r//4 -> orig f-tile 2*(r%4)+hp; wo_sb is
            # host-permuted hp-major: slot v = 4*hp + i4
            if "E" in phases:
                with (
                    tc.tile_pool(name="ps_wo", bufs=1, space="PSUM") as ps_wo,
                    tc.tile_pool(name="attg", bufs=1) as attg_pool,
                    tc.tile_pool(name="outp", bufs=4) as outp,
                ):
                    wo_ps = {}
                    for b in range(B):
                        for mt in range(SLICE // 128):
                            for oc in range(2):
                                wo_ps[b, mt, oc] = ps_wo.tile(
                                    [128, 512],
                                    dt.float32,
                                    tag=f"wo{b}{mt}{oc}",
                                    name=f"wo{b}{mt}{oc}",
                                )
                    attg = {}
                    for hp in range(2):
                        for b in range(B):
                            ag = attg_pool.tile(
                                [128, 4, SLICE],
                                dt.float8e4,
                                tag=f"ag{hp}{b}",
                                name=f"ag{hp}{b}",
                            )
                            attg[hp, b] = ag
                            nc.sync.dma_start(
                                out=ag,
                                in_=a2a_out[hp][4 * b : 4 * (b + 1), :, :].rearrange(
                                    "s (t p) q -> p (s t) q", p=128
                                ),
                            )
                    for hp in range(2):
                        for b in range(B):
                            for mt in range(SLICE // 128):
                                for oc in range(2):
                                    for u in range(2):
                                        nc.tensor.matmul(
                                            wo_ps[b, mt, oc],
                                            lhsT=attg[hp, b][
                                                :, 2 * u : 2 * u + 2,
                                                mt * 128 : (mt + 1) * 128,
                                            ],
                                            rhs=wo_sb[
                                                :,
                                                4 * hp + 2 * u : 4 * hp + 2 * u + 2,
                                                oc * 512 : (oc + 1) * 512,
                                            ],
                                            start=(hp == 0 and u == 0),
                                            stop=(hp == 1 and u == 1),
                                            perf_mode=DR,
                                        )
                    for b in range(B):
                        for mt in range(SLICE // 128):
                            for oc in range(2):
                                o_t = outp.tile([128, 512], dt.float32, tag="o")
                                # out = psum/1024 + (residual + bias)
                                nc.vector.scalar_tensor_tensor(
                                    out=o_t,
                                    in0=wo_ps[b, mt, oc],
                                    scalar=float(1.0 / (WS * WS)),
                                    in1=xrb[
                                        :,
                                        b * (SLICE // 128) + mt,
                                        oc * 512 : (oc + 1) * 512,
                                    ],
                                    op0=OP.mult,
                                    op1=OP.add,
                                )
                                nc.sync.dma_start(
                                    out=out_sl[
                                        b,
                                        mt * 128 : (mt + 1) * 128,
                                        oc * 512 : (oc + 1) * 512,
                                    ],
                                    in_=o_t,
                                )
            else:
                nc.sync.dma_start(out=out_sl[:, :, :], in_=x_res[:, :, :])

    nc.compile()
    return nc


def _get_nc(phases="ABCDE"):
    key = ("nc", phases)
    if key not in _CACHE:
        _CACHE[key] = _build(phases)
    return _CACHE[key]


def _make_in_maps(inputs):
    x = np.asarray(inputs["x"], np.float32)
    w_q = np.asarray(inputs["w_q"], np.float32)
    w_k = np.asarray(inputs["w_k"], np.float32)
    w_v = np.asarray(inputs["w_v"], np.float32)
    w_o = np.asarray(inputs["w_o"], np.float32)
    b_o = np.asarray(inputs["b_o"], np.float32)
    gamma = np.asarray(inputs["ln_gamma"], np.float32)
    beta = np.asarray(inputs["ln_beta"], np.float32)

    assert np.allclose(beta, 0.0), "nonzero ln_beta not supported"
    # hp-major f-tile permutation for DoubleRow pairing in phase E
    woT = np.ascontiguousarray(w_o.T) * WS
    woT_p = np.concatenate(
        [woT[128 * (2 * (v % 4) + v // 4) : 128 * (2 * (v % 4) + v // 4) + 128] for v in range(8)],
        axis=0,
    ).astype(F8)
    # LN gamma folds exactly into the input side of the QKV projections
    w_qg = w_q * gamma[None, :] * WS
    w_kg = w_k * gamma[None, :] * WS
    w_vg = w_v * gamma[None, :] * WS
    in_maps = []
    for r in range(NCORES):
        b, i = r // 4, r % 4
        sl = slice(DLOC * i, DLOC * (i + 1))
        in_maps.append(
            {
                "x_b": np.ascontiguousarray(x[b]),
                "wq8": np.ascontiguousarray(w_qg[sl].T).astype(F8),
                "wk8": np.ascontiguousarray(w_kg[sl].T).astype(F8),
                "wv8": np.ascontiguousarray(w_vg[sl].T).astype(F8),
                "wo8": woT_p,
                "x_res": np.ascontiguousarray(
                    x[:, SLICE * r : SLICE * (r + 1), :] + b_o[None, None, :]
                ),
            }
        )
    return in_maps


def _install_ntff_hook():
    """The agent image's antenv lacks axon_hooks; recreate it so
    trace=True can capture NTFF profiles through libaxon_pjrt.so."""
    import types

    from concourse import bass_utils

    if "antenv.axon_hooks" not in sys.modules:
        import antenv
        from trn_agent_boot.trn_boot import _ntff_profile_via_ctypes

        mod = types.ModuleType("antenv.axon_hooks")
        state = {}
        mod.set_axon_ntff_profile_hook = lambda h: state.update(h=h)
        mod.get_axon_ntff_profile_hook = lambda: state.get("h")
        sys.modules["antenv.axon_hooks"] = mod
        antenv.axon_hooks = mod
        mod.set_axon_ntff_profile_hook(
            _ntff_profile_via_ctypes("/opt/axon/libaxon_pjrt.so")
        )
    bass_utils.upload_artifacts = lambda tmpdir: tmpdir


def run(inputs, trace=False, phases="ABCDE", tmpdir=None, trace_cores=None):
    from concourse import bass_utils

    if trace:
        _install_ntff_hook()
    nc = _get_nc(phases)
    in_maps = _make_in_maps(inputs)
    res = bass_utils.run_bass_kernel_spmd(
        nc,
        in_maps,
        core_ids=list(range(NCORES)),
        trace=trace,
        tmpdir=tmpdir,
        trace_cores=trace_cores,
    )
    out = np.empty((B, S, D), np.float32)
    for r in range(NCORES):
        out[:, SLICE * r : SLICE * (r + 1), :] = res.results[r]["out_sl"]
    return out, res


def kernel(**inputs):
    out, _ = run(inputs)
    return out
